# revision 14
# baseline (speedup 1.0000x reference)
"""Trainium2 Bass kernel for GQA decode attention (nn_Attention_45844480917562).

Tensor-parallel over 8 NeuronCores: each core owns 4 query heads + 1 KV head
(wq/wk/wv column-sharded). The output projection is reduction-parallel: each
core computes its partial wo product transposed and a per-sample-group
ReduceScatter(add) leaves each core its own 512 output-feature rows; the host
only concatenates/transposes.

Compute dtype is bf16 (fp32 PSUM accumulation, fp32 softmax denominator /
division); BASS_ATTN_F32=1 switches to full fp32 at ~2x the HBM traffic.

Self-contained: hardcodes all shapes; host-side prep reshapes/transposes the
full inputs into per-core DMA-friendly layouts (K cache transposed to
[head_dim, pos], V cache chunk-major with a fused ones-column that yields the
softmax denominator for free in the P@V matmul).
"""

import os
import sys
import math

sys.path.insert(0, "/opt/trn_rl_repo")

import numpy as np
import ml_dtypes

import concourse.bass as bass
import concourse.mybir as mybir
from concourse import tile, bacc, masks
from concourse.bass_utils import run_bass_kernel_spmd

# ---------------- problem constants ----------------
DIM = 4096
N_HEADS = 32
N_KV_HEADS = 8
HEAD_DIM = 128
NCORE = 8
HPC = N_HEADS // NCORE            # 4 query heads per core
QF = HPC * HEAD_DIM               # 512 features per core
BSZ = (16, 16)
SP = (2048, 1024)                 # start_pos per group
TOT_B = 32
NFULL = (SP[0] // 128, SP[1] // 128)   # full 128-pos chunks per group: 16, 8
KCH = DIM // 128                  # 32 contraction chunks

USE_F32 = bool(int(os.environ.get("BASS_ATTN_F32", "0")))
DT = mybir.dt.float32 if USE_F32 else mybir.dt.bfloat16
NPDT = np.float32 if USE_F32 else ml_dtypes.bfloat16
SPT = 1 if USE_F32 else 2          # samples per KV tile (f32 tiles are 2x bytes)
WQ_BUFS = 2

f32 = mybir.dt.float32


def _build_nc():
    nc = bacc.Bacc(trn_type="TRN2", num_devices=NCORE, enable_asserts=True)

    # ---- I/O ----
    xh = nc.dram_tensor("xh", [128, KCH, TOT_B], DT, kind="ExternalInput")
    wqkv = nc.dram_tensor("wqkv", [128, KCH, QF + 2 * HEAD_DIM], DT, kind="ExternalInput")
    # wo in [local_c, f] layout: wo_cf[p, h, f] = wo[f, 512*r + h*128 + p]
    wo = nc.dram_tensor("wo", [128, HPC, DIM], DT, kind="ExternalInput")
    kt0 = nc.dram_tensor("kt0", [BSZ[0], 128, SP[0]], DT, kind="ExternalInput")
    kt1 = nc.dram_tensor("kt1", [BSZ[1], 128, SP[1]], DT, kind="ExternalInput")
    vp0 = nc.dram_tensor("vp0", [BSZ[0], 128, NFULL[0], 129], DT, kind="ExternalInput")
    vp1 = nc.dram_tensor("vp1", [BSZ[1], 128, NFULL[1], 129], DT, kind="ExternalInput")
    ropec = nc.dram_tensor("ropec", [128, TOT_B], f32, kind="ExternalInput")
    ropes = nc.dram_tensor("ropes", [128, TOT_B], f32, kind="ExternalInput")
    # yT: rows = this core's 512 output features (f = 512*r + row), cols = samples
    y = nc.dram_tensor("y", [QF, TOT_B], f32, kind="ExternalOutput")

    WQKV_W = QF + 2 * HEAD_DIM  # 768
    SWAP_MASK = [i ^ 1 for i in range(32)]

    with tile.TileContext(nc) as tc:
        with tc.tile_pool(name="cpool", bufs=1) as cpool, \
             tc.tile_pool(name="wpool", bufs=2) as wpool, \
             tc.tile_pool(name="kvpool", bufs=4) as kvpool, \
             tc.tile_pool(name="apool", bufs=3) as apool, \
             tc.tile_pool(name="ps_t", bufs=3, space="PSUM") as ps_t, \
             tc.tile_pool(name="dpool", bufs=1, space="DRAM") as dpool:

            # ---------- constants ----------
            ident = cpool.tile([128, 128], f32)
            masks.make_identity(nc, ident[:])
            identdt = cpool.tile([TOT_B, TOT_B], DT)
            masks.make_identity(nc, identdt[:])

            # x + wqkv go at the head of the SP ring (same ring as the KV
            # stream) so the QKV critical chain gets full DMA bandwidth
            # before the bulk KV traffic.
            x_sb = cpool.tile([128, KCH * TOT_B], DT)
            nc.sync.dma_start(x_sb[:].rearrange("p (c b) -> p c b", c=KCH), xh[:])
            ropec_sb = cpool.tile([128, TOT_B], f32)
            nc.scalar.dma_start(ropec_sb[:], ropec[:])
            ropes_sb = cpool.tile([128, TOT_B], f32)
            nc.scalar.dma_start(ropes_sb[:], ropes[:])

            # ---------- phase A: QKV projection ----------
            with tc.tile_pool(name="ps_a", bufs=1, space="PSUM") as ps_a:
                qkv_ps = ps_a.tile([TOT_B, WQKV_W], f32)
                for P in range(4):
                    wq_t = wpool.tile([128, 8 * WQKV_W], DT, tag="wq", bufs=WQ_BUFS)
                    nc.sync.dma_start(
                        wq_t[:].rearrange("p (c j) -> p c j", c=8),
                        wqkv[:, 8 * P:8 * P + 8, :],
                    )
                    for ci in range(8):
                        c = 8 * P + ci
                        lhs = x_sb[:, TOT_B * c:TOT_B * (c + 1)]
                        rhs = wq_t[:, WQKV_W * ci:WQKV_W * (ci + 1)]
                        nc.tensor.matmul(qkv_ps[:, 0:512], lhs, rhs[:, 0:512],
                                         start=(c == 0), stop=(c == KCH - 1))
                        nc.tensor.matmul(qkv_ps[:, 512:768], lhs, rhs[:, 512:768],
                                         start=(c == 0), stop=(c == KCH - 1))

                qkv_sb = cpool.tile([TOT_B, WQKV_W], f32)
                nc.scalar.copy(qkv_sb[:], qkv_ps[:])

            # wo weights prefetch tile. Issued on the SP ring between wqkv and
            # the KV stream: PE's in-order SEQ hits the first group's
            # partial-wo matmuls right after that group's attention, so wo
            # must be resident by ~55us or PE stalls and the second group's
            # whole pipeline backs up. (Not at the very top: a single 4.2MB
            # DMACopy would hold the shared DMA pool ahead of wqkv and delay
            # the QKV projection that gates all attention.)
            wo_all = wpool.tile([128, KCH * QF], DT, tag="wo", bufs=1)
            nc.sync.dma_start(
                wo_all[:].rearrange("p (c j) -> p c j", c=HPC), wo[:])

            # new-position V (plus ones column for the softmax denominator)
            vnew = cpool.tile([TOT_B, 129], DT)
            nc.vector.tensor_copy(vnew[:, 0:HEAD_DIM], qkv_sb[:, 640:768])
            nc.vector.memset(vnew[:, 128:129], 1.0)

            # ---------- transpose q heads + k, apply RoPE ----------
            qT4 = cpool.tile([128, HPC * TOT_B], DT)   # col = b*4 + h
            kTn = cpool.tile([128, TOT_B], DT)         # col = b
            for h in range(HPC + 1):                   # 4 q heads then k
                tp = ps_t.tile([128, TOT_B], f32, tag="tp")
                nc.tensor.transpose(tp[:], qkv_sb[:, 128 * h:128 * (h + 1)],
                                    ident[0:TOT_B, 0:TOT_B])
                t_sb = apool.tile([128, TOT_B], f32, tag="tr")
                nc.vector.tensor_copy(t_sb[:], tp[:])
                sw = apool.tile([128, TOT_B], f32, tag="sw")
                nc.vector.stream_shuffle(sw[:], t_sb[:], SWAP_MASK)
                t1 = apool.tile([128, TOT_B], f32, tag="t1")
                nc.vector.tensor_mul(t1[:], t_sb[:], ropec_sb[:])
                nc.vector.tensor_mul(sw[:], sw[:], ropes_sb[:])
                if h < HPC:
                    dest = qT4[:, h::HPC]
                else:
                    dest = kTn[:]
                nc.vector.tensor_add(dest, t1[:], sw[:])

            # ---------- phase B: attention over the KV cache ----------
            attnT = cpool.tile([128, HPC * TOT_B], DT)  # col = h*32 + b
            kts = (kt0, kt1)
            vps = (vp0, vp1)
            # Output projection is reduction-parallel: each core computes its
            # partial wo product (transposed, [4096, 16] per sample group) and
            # a ReduceScatter(add) sums across cores, leaving each core its
            # own 512 output-feature rows. Group 1 (1024-pos, half the KV
            # bytes) goes FIRST so its collective fires ~54us in, fully
            # overlapped by group 0's KV stream; only group 0's collective
            # sits in the tail. COLLECTIVE_CORES is exclusive, so two
            # back-to-back collectives at the tail would serialize 2x15.8us.
            rs_in = [dpool.tile([DIM, 16], f32, name=f"rs_in{g}") for g in range(2)]
            rs_out = [dpool.tile([QF, 16], f32, name=f"rs_out{g}")
                      for g in range(2)]
            with tc.tile_pool(name="ps_b", bufs=2, space="PSUM") as ps_b:
                for gi, g in enumerate((1, 0)):
                    npos = SP[g]
                    nf = NFULL[g]
                    ncol = 4 * nf
                    vw = 129 * nf
                    # Small KV slices keep the shared DMA resource's FIFO
                    # shallow: rs_in/y writes and the collectives' inputs
                    # would otherwise queue behind multiple megabyte-sized KV
                    # reads (observed +16us on the first collective). Taper
                    # the tail of the LAST group so the serial per-sample
                    # attention chain after the final DMA byte is short.
                    if SPT == 1:
                        blocks = [1] * BSZ[g]
                    elif gi == 1:
                        blocks = [2] * 7 + [1, 1]
                    else:
                        blocks = [2] * (BSZ[g] // 2)
                    s_off = 0
                    for blk in blocks:
                        ktile = kvpool.tile([128, SPT * SP[0]], DT, tag="kt")
                        nc.sync.dma_start(
                            ktile[:, 0:blk * npos].rearrange("p (s n) -> p s n", s=blk),
                            kts[g][s_off:s_off + blk].rearrange("s p n -> p s n"),
                        )
                        vtile = kvpool.tile([128, SPT * 129 * NFULL[0]], DT, tag="vt")
                        nc.sync.dma_start(
                            vtile[:, 0:blk * vw].rearrange("p (s c d) -> p s c d", s=blk, c=nf),
                            vps[g][s_off:s_off + blk].rearrange("s p c d -> p s c d"),
                        )
                        for j in range(blk):
                            b = 16 * g + s_off + j
                            ks = ktile[:, j * npos:(j + 1) * npos]
                            vs = vtile[:, j * vw:(j + 1) * vw]
                            q_b = qT4[:, HPC * b:HPC * (b + 1)]

                            sc_ps = ps_b.tile([128, 68], f32, tag="sc")
                            for c in range(nf):
                                nc.tensor.matmul(sc_ps[:, 4 * c:4 * c + 4],
                                                 ks[:, 128 * c:128 * (c + 1)], q_b,
                                                 start=True, stop=True)
                            nc.tensor.matmul(sc_ps[0:1, ncol:ncol + 4],
                                             kTn[:, b:b + 1], q_b,
                                             start=True, stop=True)

                            pr = apool.tile([128, 68], DT, tag="pr")
                            nc.scalar.activation(pr[:, 0:ncol], sc_ps[:, 0:ncol],
                                                 mybir.ActivationFunctionType.Exp)
                            nc.scalar.activation(pr[0:1, ncol:ncol + 4],
                                                 sc_ps[0:1, ncol:ncol + 4],
                                                 mybir.ActivationFunctionType.Exp)

                            # select row b of vnew into partition 0 (psum), for the
                            # tail matmul rhs (moving operand must be partition-0 based)
                            vrow_ps = ps_b.tile([1, 129], f32, tag="vr", bufs=1)
                            nc.tensor.matmul(vrow_ps[:], identdt[:, b:b + 1], vnew[:],
                                             start=True, stop=True)
                            vrow = apool.tile([1, 129], DT, tag="vrow")
                            nc.vector.tensor_copy(vrow[:], vrow_ps[:])

                            o_ps = ps_b.tile([HPC, 129], f32, tag="o")
                            for c in range(nf):
                                nc.tensor.matmul(o_ps[:], pr[:, 4 * c:4 * c + 4],
                                                 vs[:, 129 * c:129 * (c + 1)],
                                                 start=(c == 0), stop=False)
                            nc.tensor.matmul(o_ps[:], pr[0:1, ncol:ncol + 4],
                                             vrow[:], start=False, stop=True)

                            rec = apool.tile([HPC, 1], f32, tag="rec")
                            nc.vector.reciprocal(rec[:], o_ps[:, 128:129])
                            at = apool.tile([HPC, HEAD_DIM], f32, tag="at")
                            nc.vector.tensor_scalar_mul(at[:], o_ps[:, 0:HEAD_DIM], rec[:])

                            tp2 = ps_t.tile([128, TOT_B], f32, tag="tp")
                            nc.tensor.transpose(tp2[:, 0:HPC], at[:], ident[0:HPC, 0:HPC])
                            nc.vector.tensor_copy(attnT[:, b::TOT_B], tp2[:, 0:HPC])
                        s_off += blk

                    # this group's samples are done: partial wo product
                    # partialT[f, b] = sum_c wo[f, c] * attn[b, c]  (c = own features)
                    pT_sb = apool.tile([128, 32 * 16], f32, tag="pt", bufs=2)
                    for fq in range(4):          # 8 fb blocks per PSUM bank
                        pt_ps = ps_t.tile([128, 128], f32, tag="tp")
                        for fi in range(8):
                            fb = 8 * fq + fi
                            for h in range(HPC):
                                nc.tensor.matmul(
                                    pt_ps[:, 16 * fi:16 * (fi + 1)],
                                    wo_all[:, h * DIM + 128 * fb:h * DIM + 128 * (fb + 1)],
                                    attnT[:, TOT_B * h + 16 * g:TOT_B * h + 16 * (g + 1)],
                                    start=(h == 0), stop=(h == HPC - 1))
                        nc.vector.tensor_copy(pT_sb[:, 128 * fq:128 * (fq + 1)], pt_ps[:])
                        # rs_in row order is permuted within each rank's
                        # 512-row block (row = 512r + 4p + fb%4) so DRAM
                        # writes are 256B contiguous runs instead of 64B;
                        # host un-permutes (the global mapping rank=fb//4,
                        # row-in-rank=4p+fb%4 is chunk-size invariant).
                        # The tail group ships per-fq quarters on the idle ACT
                        # HWDGE ring so its collective fires ASAP after the
                        # last partial; the first group ships halves on
                        # gpsimd mid-stream.
                        if gi == 1:
                            nc.scalar.dma_start(
                                rs_in[g][1024 * fq:1024 * (fq + 1)].rearrange(
                                    "(r p four) b -> p r (four b)", r=2, four=HPC),
                                pT_sb[:, 128 * fq:128 * (fq + 1)].rearrange(
                                    "p (r four b) -> p r (four b)", r=2, four=HPC),
                            )
                        elif fq == 1 or fq == 3:
                            half = fq // 2
                            nc.gpsimd.dma_start(
                                rs_in[g][2048 * half:2048 * (half + 1)].rearrange(
                                    "(r p four) b -> p r (four b)", r=NCORE // 2, four=HPC),
                                pT_sb[:, 256 * half:256 * (half + 1)].rearrange(
                                    "p (r four b) -> p r (four b)", r=NCORE // 2, four=HPC),
                            )
                    nc.gpsimd.collective_compute(
                        "ReduceScatter", mybir.AluOpType.add,
                        replica_groups=[list(range(NCORE))],
                        ins=[rs_in[g].opt()], outs=[rs_out[g].opt()],
                    )

                # y writebacks go on the SP ring AFTER both groups' KV
                # dma_starts are queued: a y write waits on its collective,
                # and anything queued behind it on the same in-order ring
                # would stall until the collective completes.
                for g in (1, 0):
                    nc.sync.dma_start(y[:, 16 * g:16 * (g + 1)], rs_out[g][:])

    nc.finalize()
    return nc


_NC_CACHE = None


def _get_nc():
    global _NC_CACHE
    if _NC_CACHE is None:
        _NC_CACHE = _build_nc()
    return _NC_CACHE


def _prep_inputs(inputs):
    """Shard + lay out the full inputs for the 8 cores."""
    x = np.asarray(inputs["x"], np.float32)
    wq = np.asarray(inputs["wq"], np.float32)
    wk = np.asarray(inputs["wk"], np.float32)
    wv = np.asarray(inputs["wv"], np.float32)
    wo = np.asarray(inputs["wo"], np.float32)
    fc = np.asarray(inputs["freqs_cos"], np.float32)
    fs = np.asarray(inputs["freqs_sin"], np.float32)
    caches = (
        (np.asarray(inputs["cache_k0"], np.float32), np.asarray(inputs["cache_v0"], np.float32)),
        (np.asarray(inputs["cache_k1"], np.float32), np.asarray(inputs["cache_v1"], np.float32)),
    )

    x_flat = x.reshape(TOT_B, DIM)
    xh = np.ascontiguousarray(
        x_flat.T.reshape(KCH, 128, TOT_B).transpose(1, 0, 2)
    ).astype(NPDT)

    # RoPE tables: per-column position (2048 for tokens 0-15, 1024 for 16-31)
    C = np.empty((128, TOT_B), np.float32)
    S = np.empty((128, TOT_B), np.float32)
    for g in range(2):
        cos = fc[SP[g]]
        sin = fs[SP[g]]
        cols = slice(16 * g, 16 * (g + 1))
        C[0::2, cols] = cos[:, None]
        C[1::2, cols] = cos[:, None]
        S[0::2, cols] = -sin[:, None]
        S[1::2, cols] = sin[:, None]

    scale = 1.0 / math.sqrt(HEAD_DIM)

    def _prep_core(r):
        w_q = wq[QF * r:QF * (r + 1)] * scale
        w_k = wk[HEAD_DIM * r:HEAD_DIM * (r + 1)]
        w_v = wv[HEAD_DIM * r:HEAD_DIM * (r + 1)]
        wqkvT = np.concatenate([w_q, w_k, w_v], axis=0).T  # [4096, 768]
        wqkv_hp = np.ascontiguousarray(
            wqkvT.reshape(KCH, 128, 768).transpose(1, 0, 2)
        ).astype(NPDT)

        # wo_cf[local_c, f] = wo[f, 512r + local_c]  -> [128, HPC, 4096]
        wo_cf = wo[:, QF * r:QF * (r + 1)].T  # [512, 4096]
        wo_hp = np.ascontiguousarray(
            wo_cf.reshape(HPC, 128, DIM).transpose(1, 0, 2)
        ).astype(NPDT)

        m = {"xh": xh, "wqkv": wqkv_hp, "wo": wo_hp,
             "ropec": C, "ropes": S}
        for g in range(2):
            ck, cv = caches[g]
            npos = SP[g]
            nf = NFULL[g]
            # cast to the wire dtype first, then do the layout copy at half width
            kslab = ck[:, :npos, r, :].astype(NPDT)       # [16, npos, 128]
            kt = np.ascontiguousarray(kslab.transpose(0, 2, 1))  # [16, 128, npos]
            vslab = cv[:, :npos, r, :].astype(NPDT).reshape(BSZ[g], nf, 128, HEAD_DIM)
            vp = np.empty((BSZ[g], 128, nf, 129), NPDT)
            vp[:, :, :, HEAD_DIM] = NPDT(1.0)
            vp[:, :, :, :HEAD_DIM] = vslab.transpose(0, 2, 1, 3)
            m[f"kt{g}"] = kt
            m[f"vp{g}"] = vp
        return m

    from concurrent.futures import ThreadPoolExecutor
    with ThreadPoolExecutor(max_workers=NCORE) as ex:
        in_maps = list(ex.map(_prep_core, range(NCORE)))
    return in_maps


def _run(inputs, trace=False):
    nc = _get_nc()
    in_maps = _prep_inputs(inputs)
    res = run_bass_kernel_spmd(nc, in_maps, core_ids=list(range(NCORE)), trace=trace)
    # each core returns yT rows [512r : 512r+512] of the [4096, 32] output,
    # row-permuted within the block (row = 4p + fb%4 -> f_local = 128*(fb%4) + p)
    parts = []
    for r in range(NCORE):
        yr = res.results[r]["y"]  # [512, 32]
        parts.append(yr.reshape(128, HPC, TOT_B).transpose(1, 0, 2).reshape(QF, TOT_B))
    y_t = np.concatenate(parts, axis=0)
    out = np.ascontiguousarray(y_t.T).reshape(TOT_B, 1, DIM).astype(np.float32)
    return out, res


def kernel(**inputs):
    try:
        out, _ = _run(inputs, trace=False)
    except Exception:
        # transient NRT/axon hiccups have been observed to recover on retry
        out, _ = _run(inputs, trace=False)
    return out



# revision 24
# speedup vs baseline: 1.0127x; 1.0127x over previous
"""Trainium2 Bass kernel for GQA decode attention (nn_Attention_45844480917562).

Tensor-parallel over 8 NeuronCores: each core owns 4 query heads + 1 KV head
(wq/wk/wv column-sharded). The output projection is reduction-parallel: each
core computes its partial wo product transposed and a per-sample-group
ReduceScatter(add) leaves each core its own 512 output-feature rows; the host
only concatenates/transposes.

Compute dtype is bf16 (fp32 PSUM accumulation, fp32 softmax denominator /
division); BASS_ATTN_F32=1 switches to full fp32 at ~2x the HBM traffic.

Self-contained: hardcodes all shapes; host-side prep reshapes/transposes the
full inputs into per-core DMA-friendly layouts (K cache transposed to
[head_dim, pos], V cache chunk-major with a fused ones-column that yields the
softmax denominator for free in the P@V matmul).
"""

import os
import sys
import math

sys.path.insert(0, "/opt/trn_rl_repo")

import numpy as np
import ml_dtypes

import concourse.bass as bass
import concourse.mybir as mybir
from concourse import tile, bacc, masks
from concourse.bass_utils import run_bass_kernel_spmd

# ---------------- problem constants ----------------
DIM = 4096
N_HEADS = 32
N_KV_HEADS = 8
HEAD_DIM = 128
NCORE = 8
HPC = N_HEADS // NCORE            # 4 query heads per core
QF = HPC * HEAD_DIM               # 512 features per core
BSZ = (16, 16)
SP = (2048, 1024)                 # start_pos per group
TOT_B = 32
NFULL = (SP[0] // 128, SP[1] // 128)   # full 128-pos chunks per group: 16, 8
KCH = DIM // 128                  # 32 contraction chunks

USE_F32 = bool(int(os.environ.get("BASS_ATTN_F32", "0")))
DT = mybir.dt.float32 if USE_F32 else mybir.dt.bfloat16
NPDT = np.float32 if USE_F32 else ml_dtypes.bfloat16
SPT = 1 if USE_F32 else 2          # samples per KV tile (f32 tiles are 2x bytes)
WQ_BUFS = int(os.environ.get("BASS_WQ_BUFS", "4"))

f32 = mybir.dt.float32


def _build_nc():
    nc = bacc.Bacc(trn_type="TRN2", num_devices=NCORE, enable_asserts=True)

    # ---- I/O ----
    xh = nc.dram_tensor("xh", [128, KCH, TOT_B], DT, kind="ExternalInput")
    wqkv = nc.dram_tensor("wqkv", [128, KCH, QF + 2 * HEAD_DIM], DT, kind="ExternalInput")
    # wo in [local_c, f] layout: wo_cf[p, h, f] = wo[f, 512*r + h*128 + p]
    wo = nc.dram_tensor("wo", [128, HPC, DIM], DT, kind="ExternalInput")
    kt0 = nc.dram_tensor("kt0", [BSZ[0], 128, SP[0]], DT, kind="ExternalInput")
    kt1 = nc.dram_tensor("kt1", [BSZ[1], 128, SP[1]], DT, kind="ExternalInput")
    vp0 = nc.dram_tensor("vp0", [BSZ[0], 128, NFULL[0], 129], DT, kind="ExternalInput")
    vp1 = nc.dram_tensor("vp1", [BSZ[1], 128, NFULL[1], 129], DT, kind="ExternalInput")
    ropec = nc.dram_tensor("ropec", [128, TOT_B], f32, kind="ExternalInput")
    ropes = nc.dram_tensor("ropes", [128, TOT_B], f32, kind="ExternalInput")
    # yT: rows = this core's 512 output features (f = 512*r + row), cols = samples
    y = nc.dram_tensor("y", [QF, TOT_B], f32, kind="ExternalOutput")

    WQKV_W = QF + 2 * HEAD_DIM  # 768
    SWAP_MASK = [i ^ 1 for i in range(32)]

    with tile.TileContext(nc) as tc:
        with tc.tile_pool(name="cpool", bufs=1) as cpool, \
             tc.tile_pool(name="wpool", bufs=2) as wpool, \
             tc.tile_pool(name="kvpool", bufs=int(os.environ.get("BASS_KV_BUFS", "3"))) as kvpool, \
             tc.tile_pool(name="apool", bufs=3) as apool, \
             tc.tile_pool(name="ps_t", bufs=3, space="PSUM") as ps_t, \
             tc.tile_pool(name="dpool", bufs=1, space="DRAM") as dpool:

            # ---------- constants ----------
            ident = cpool.tile([128, 128], f32)
            masks.make_identity(nc, ident[:])

            # x + wqkv go at the head of the SP ring (same ring as the KV
            # stream) so the QKV critical chain gets full DMA bandwidth
            # before the bulk KV traffic.
            x_sb = cpool.tile([128, KCH * TOT_B], DT)
            nc.sync.dma_start(x_sb[:].rearrange("p (c b) -> p c b", c=KCH), xh[:])
            ropec_sb = cpool.tile([128, TOT_B], f32)
            nc.scalar.dma_start(ropec_sb[:], ropec[:])
            ropes_sb = cpool.tile([128, TOT_B], f32)
            nc.scalar.dma_start(ropes_sb[:], ropes[:])

            # ---------- phase A: QKV projection ----------
            with tc.tile_pool(name="ps_a", bufs=1, space="PSUM") as ps_a:
                qkv_ps = ps_a.tile([TOT_B, WQKV_W], f32)
                for P in range(4):
                    wq_t = wpool.tile([128, 8 * WQKV_W], DT, tag="wq", bufs=WQ_BUFS)
                    nc.sync.dma_start(
                        wq_t[:].rearrange("p (c j) -> p c j", c=8),
                        wqkv[:, 8 * P:8 * P + 8, :],
                    )
                    for ci in range(8):
                        c = 8 * P + ci
                        lhs = x_sb[:, TOT_B * c:TOT_B * (c + 1)]
                        rhs = wq_t[:, WQKV_W * ci:WQKV_W * (ci + 1)]
                        nc.tensor.matmul(qkv_ps[:, 0:512], lhs, rhs[:, 0:512],
                                         start=(c == 0), stop=(c == KCH - 1))
                        nc.tensor.matmul(qkv_ps[:, 512:768], lhs, rhs[:, 512:768],
                                         start=(c == 0), stop=(c == KCH - 1))

                qkv_sb = cpool.tile([TOT_B, WQKV_W], f32)
                nc.scalar.copy(qkv_sb[:], qkv_ps[:])

            # wo weights prefetch tile. Issued on the SP ring between wqkv and
            # the KV stream: PE's in-order SEQ hits the first group's
            # partial-wo matmuls right after that group's attention, so wo
            # must be resident by ~55us or PE stalls and the second group's
            # whole pipeline backs up. (Not at the very top: a single 4.2MB
            # DMACopy would hold the shared DMA pool ahead of wqkv and delay
            # the QKV projection that gates all attention.)
            wo_all = wpool.tile([128, KCH * QF], DT, tag="wo", bufs=1)
            nc.sync.dma_start(
                wo_all[:].rearrange("p (c j) -> p c j", c=HPC), wo[:])

            # new-position V (plus ones column for the softmax denominator),
            # then flattened onto partition 0 via one SBUF->SBUF DMA so each
            # sample's row is directly usable as the PV-tail moving operand
            # (no per-sample select matmul on the critical chain).
            vnew = cpool.tile([TOT_B, 129], DT)
            nc.vector.tensor_copy(vnew[:, 0:HEAD_DIM], qkv_sb[:, 640:768])
            nc.vector.memset(vnew[:, 128:129], 1.0)
            vrow_all = cpool.tile([1, TOT_B * 129], DT)
            nc.gpsimd.dma_start(
                vrow_all[:].rearrange("p (b j) -> p b j", b=TOT_B), vnew[:])

            # ---------- transpose q heads + k, apply RoPE ----------
            qT4 = cpool.tile([128, HPC * TOT_B], DT)   # col = b*4 + h
            kTn = cpool.tile([128, TOT_B], DT)         # col = b
            for h in range(HPC + 1):                   # 4 q heads then k
                tp = ps_t.tile([128, TOT_B], f32, tag="tp")
                nc.tensor.transpose(tp[:], qkv_sb[:, 128 * h:128 * (h + 1)],
                                    ident[0:TOT_B, 0:TOT_B])
                t_sb = apool.tile([128, TOT_B], f32, tag="tr")
                nc.vector.tensor_copy(t_sb[:], tp[:])
                sw = apool.tile([128, TOT_B], f32, tag="sw")
                nc.vector.stream_shuffle(sw[:], t_sb[:], SWAP_MASK)
                t1 = apool.tile([128, TOT_B], f32, tag="t1")
                nc.vector.tensor_mul(t1[:], t_sb[:], ropec_sb[:])
                nc.vector.tensor_mul(sw[:], sw[:], ropes_sb[:])
                if h < HPC:
                    dest = qT4[:, h::HPC]
                else:
                    dest = kTn[:]
                nc.vector.tensor_add(dest, t1[:], sw[:])

            # ---------- phase B: attention over the KV cache ----------
            attnT = cpool.tile([128, HPC * TOT_B], DT)  # col = h*32 + b
            kts = (kt0, kt1)
            vps = (vp0, vp1)
            # Output projection is reduction-parallel: each core computes its
            # partial wo product (transposed, [4096, 16] per sample group) and
            # a ReduceScatter(add) sums across cores, leaving each core its
            # own 512 output-feature rows. Group 1 (1024-pos, half the KV
            # bytes) goes FIRST so its collective fires ~54us in, fully
            # overlapped by group 0's KV stream; only group 0's collective
            # sits in the tail. COLLECTIVE_CORES is exclusive, so two
            # back-to-back collectives at the tail would serialize 2x15.8us.
            rs_in = [dpool.tile([DIM, 16], f32, name=f"rs_in{g}") for g in range(2)]
            rs_out = [dpool.tile([QF, 16], f32, name=f"rs_out{g}")
                      for g in range(2)]
            with tc.tile_pool(name="ps_b", bufs=2, space="PSUM") as ps_b:
                for gi, g in enumerate((1, 0)):
                    npos = SP[g]
                    nf = NFULL[g]
                    ncol = 4 * nf
                    vw = 129 * nf
                    # Small KV slices keep the shared DMA resource's FIFO
                    # shallow: rs_in/y writes and the collectives' inputs
                    # would otherwise queue behind multiple megabyte-sized KV
                    # reads (observed +16us on the first collective). Taper
                    # the tail of the LAST group so the serial per-sample
                    # attention chain after the final DMA byte is short.
                    if SPT == 1:
                        blocks = [1] * BSZ[g]
                    elif gi == 1:
                        blocks = [2] * 7 + [1, 1]
                    else:
                        blocks = [2] * (BSZ[g] // 2)
                    s_off = 0
                    for blk in blocks:
                        ktile = kvpool.tile([128, SPT * SP[0]], DT, tag="kt")
                        nc.sync.dma_start(
                            ktile[:, 0:blk * npos].rearrange("p (s n) -> p s n", s=blk),
                            kts[g][s_off:s_off + blk].rearrange("s p n -> p s n"),
                        )
                        vtile = kvpool.tile([128, SPT * 129 * NFULL[0]], DT, tag="vt")
                        nc.sync.dma_start(
                            vtile[:, 0:blk * vw].rearrange("p (s c d) -> p s c d", s=blk, c=nf),
                            vps[g][s_off:s_off + blk].rearrange("s p c d -> p s c d"),
                        )
                        for j in range(blk):
                            b = 16 * g + s_off + j
                            ks = ktile[:, j * npos:(j + 1) * npos]
                            vs = vtile[:, j * vw:(j + 1) * vw]
                            q_b = qT4[:, HPC * b:HPC * (b + 1)]

                            sc_ps = ps_b.tile([128, 68], f32, tag="sc")
                            for c in range(nf):
                                nc.tensor.matmul(sc_ps[:, 4 * c:4 * c + 4],
                                                 ks[:, 128 * c:128 * (c + 1)], q_b,
                                                 start=True, stop=True)
                            nc.tensor.matmul(sc_ps[0:1, ncol:ncol + 4],
                                             kTn[:, b:b + 1], q_b,
                                             start=True, stop=True)

                            # one exp over the scores plus the new-position
                            # row (cols ncol:ncol+4 rows 1.. hold stale psum
                            # floats that are exp'd but never read)
                            pr = apool.tile([128, 68], DT, tag="pr")
                            nc.scalar.activation(pr[:, 0:ncol + 4],
                                                 sc_ps[:, 0:ncol + 4],
                                                 mybir.ActivationFunctionType.Exp)

                            o_ps = ps_b.tile([HPC, 129], f32, tag="o")
                            for c in range(nf):
                                nc.tensor.matmul(o_ps[:], pr[:, 4 * c:4 * c + 4],
                                                 vs[:, 129 * c:129 * (c + 1)],
                                                 start=(c == 0), stop=False)
                            nc.tensor.matmul(o_ps[:], pr[0:1, ncol:ncol + 4],
                                             vrow_all[0:1, 129 * b:129 * (b + 1)],
                                             start=False, stop=True)

                            rec = apool.tile([HPC, 1], f32, tag="rec")
                            nc.vector.reciprocal(rec[:], o_ps[:, 128:129])
                            at = apool.tile([HPC, HEAD_DIM], f32, tag="at")
                            nc.vector.tensor_scalar_mul(at[:], o_ps[:, 0:HEAD_DIM], rec[:])

                            tp2 = ps_t.tile([128, TOT_B], f32, tag="tp")
                            nc.tensor.transpose(tp2[:, 0:HPC], at[:], ident[0:HPC, 0:HPC])
                            nc.vector.tensor_copy(attnT[:, b::TOT_B], tp2[:, 0:HPC])
                        s_off += blk

                    # this group's samples are done: partial wo product
                    # partialT[f, b] = sum_c wo[f, c] * attn[b, c]  (c = own features)
                    pT_sb = apool.tile([128, 32 * 16], f32, tag="pt", bufs=2)
                    for fq in range(4):          # 8 fb blocks per PSUM bank
                        pt_ps = ps_t.tile([128, 128], f32, tag="tp")
                        for fi in range(8):
                            fb = 8 * fq + fi
                            for h in range(HPC):
                                nc.tensor.matmul(
                                    pt_ps[:, 16 * fi:16 * (fi + 1)],
                                    wo_all[:, h * DIM + 128 * fb:h * DIM + 128 * (fb + 1)],
                                    attnT[:, TOT_B * h + 16 * g:TOT_B * h + 16 * (g + 1)],
                                    start=(h == 0), stop=(h == HPC - 1))
                        nc.vector.tensor_copy(pT_sb[:, 128 * fq:128 * (fq + 1)], pt_ps[:])
                        # rs_in row order is permuted within each rank's
                        # 512-row block (row = 512r + 4p + fb%4) so DRAM
                        # writes are 256B contiguous runs instead of 64B;
                        # host un-permutes (the global mapping rank=fb//4,
                        # row-in-rank=4p+fb%4 is chunk-size invariant).
                        # Shipped in two rank-halves; the tail group's go on
                        # the idle ACT HWDGE ring (faster first-byte than
                        # SWDGE), the first group's on gpsimd mid-stream.
                        if fq == 1 or fq == 3:
                            half = fq // 2
                            dma_eng = nc.scalar if gi == 1 else nc.gpsimd
                            dma_eng.dma_start(
                                rs_in[g][2048 * half:2048 * (half + 1)].rearrange(
                                    "(r p four) b -> p r (four b)", r=NCORE // 2, four=HPC),
                                pT_sb[:, 256 * half:256 * (half + 1)].rearrange(
                                    "p (r four b) -> p r (four b)", r=NCORE // 2, four=HPC),
                            )
                    nc.gpsimd.collective_compute(
                        "ReduceScatter", mybir.AluOpType.add,
                        replica_groups=[list(range(NCORE))],
                        ins=[rs_in[g].opt()], outs=[rs_out[g].opt()],
                    )

                # y writebacks go on the SP ring AFTER both groups' KV
                # dma_starts are queued: a y write waits on its collective,
                # and anything queued behind it on the same in-order ring
                # would stall until the collective completes.
                for g in (1, 0):
                    nc.sync.dma_start(y[:, 16 * g:16 * (g + 1)], rs_out[g][:])

    nc.finalize()
    return nc


_NC_CACHE = None


def _get_nc():
    global _NC_CACHE
    if _NC_CACHE is None:
        _NC_CACHE = _build_nc()
    return _NC_CACHE


def _prep_inputs(inputs):
    """Shard + lay out the full inputs for the 8 cores."""
    x = np.asarray(inputs["x"], np.float32)
    wq = np.asarray(inputs["wq"], np.float32)
    wk = np.asarray(inputs["wk"], np.float32)
    wv = np.asarray(inputs["wv"], np.float32)
    wo = np.asarray(inputs["wo"], np.float32)
    fc = np.asarray(inputs["freqs_cos"], np.float32)
    fs = np.asarray(inputs["freqs_sin"], np.float32)
    caches = (
        (np.asarray(inputs["cache_k0"], np.float32), np.asarray(inputs["cache_v0"], np.float32)),
        (np.asarray(inputs["cache_k1"], np.float32), np.asarray(inputs["cache_v1"], np.float32)),
    )

    x_flat = x.reshape(TOT_B, DIM)
    xh = np.ascontiguousarray(
        x_flat.T.reshape(KCH, 128, TOT_B).transpose(1, 0, 2)
    ).astype(NPDT)

    # RoPE tables: per-column position (2048 for tokens 0-15, 1024 for 16-31)
    C = np.empty((128, TOT_B), np.float32)
    S = np.empty((128, TOT_B), np.float32)
    for g in range(2):
        cos = fc[SP[g]]
        sin = fs[SP[g]]
        cols = slice(16 * g, 16 * (g + 1))
        C[0::2, cols] = cos[:, None]
        C[1::2, cols] = cos[:, None]
        S[0::2, cols] = -sin[:, None]
        S[1::2, cols] = sin[:, None]

    scale = 1.0 / math.sqrt(HEAD_DIM)

    def _prep_core(r):
        w_q = wq[QF * r:QF * (r + 1)] * scale
        w_k = wk[HEAD_DIM * r:HEAD_DIM * (r + 1)]
        w_v = wv[HEAD_DIM * r:HEAD_DIM * (r + 1)]
        wqkvT = np.concatenate([w_q, w_k, w_v], axis=0).T  # [4096, 768]
        wqkv_hp = np.ascontiguousarray(
            wqkvT.reshape(KCH, 128, 768).transpose(1, 0, 2)
        ).astype(NPDT)

        # wo_cf[local_c, f] = wo[f, 512r + local_c]  -> [128, HPC, 4096]
        wo_cf = wo[:, QF * r:QF * (r + 1)].T  # [512, 4096]
        wo_hp = np.ascontiguousarray(
            wo_cf.reshape(HPC, 128, DIM).transpose(1, 0, 2)
        ).astype(NPDT)

        m = {"xh": xh, "wqkv": wqkv_hp, "wo": wo_hp,
             "ropec": C, "ropes": S}
        for g in range(2):
            ck, cv = caches[g]
            npos = SP[g]
            nf = NFULL[g]
            # cast to the wire dtype first, then do the layout copy at half width
            kslab = ck[:, :npos, r, :].astype(NPDT)       # [16, npos, 128]
            kt = np.ascontiguousarray(kslab.transpose(0, 2, 1))  # [16, 128, npos]
            vslab = cv[:, :npos, r, :].astype(NPDT).reshape(BSZ[g], nf, 128, HEAD_DIM)
            vp = np.empty((BSZ[g], 128, nf, 129), NPDT)
            vp[:, :, :, HEAD_DIM] = NPDT(1.0)
            vp[:, :, :, :HEAD_DIM] = vslab.transpose(0, 2, 1, 3)
            m[f"kt{g}"] = kt
            m[f"vp{g}"] = vp
        return m

    from concurrent.futures import ThreadPoolExecutor
    with ThreadPoolExecutor(max_workers=NCORE) as ex:
        in_maps = list(ex.map(_prep_core, range(NCORE)))
    return in_maps


def _run(inputs, trace=False):
    nc = _get_nc()
    in_maps = _prep_inputs(inputs)
    res = run_bass_kernel_spmd(nc, in_maps, core_ids=list(range(NCORE)), trace=trace)
    # each core returns yT rows [512r : 512r+512] of the [4096, 32] output,
    # row-permuted within the block (row = 4p + fb%4 -> f_local = 128*(fb%4) + p)
    parts = []
    for r in range(NCORE):
        yr = res.results[r]["y"]  # [512, 32]
        parts.append(yr.reshape(128, HPC, TOT_B).transpose(1, 0, 2).reshape(QF, TOT_B))
    y_t = np.concatenate(parts, axis=0)
    out = np.ascontiguousarray(y_t.T).reshape(TOT_B, 1, DIM).astype(np.float32)
    return out, res


def kernel(**inputs):
    try:
        out, _ = _run(inputs, trace=False)
    except Exception:
        # transient NRT/axon hiccups have been observed to recover on retry
        out, _ = _run(inputs, trace=False)
    return out



# revision 35
# speedup vs baseline: 1.0308x; 1.0178x over previous
"""Trainium2 Bass kernel for GQA decode attention (nn_Attention_45844480917562).

Tensor-parallel over 8 NeuronCores: each core owns 4 query heads + 1 KV head
(wq/wk/wv column-sharded). The output projection is reduction-parallel: each
core computes its partial wo product transposed and a per-sample-group
ReduceScatter(add) leaves each core its own 512 output-feature rows; the host
only concatenates/transposes.

Compute dtype is bf16 (fp32 PSUM accumulation, fp32 softmax denominator /
division); BASS_ATTN_F32=1 switches to full fp32 at ~2x the HBM traffic.

Self-contained: hardcodes all shapes; host-side prep reshapes/transposes the
full inputs into per-core DMA-friendly layouts (K cache transposed to
[head_dim, pos], V cache chunk-major with a fused ones-column that yields the
softmax denominator for free in the P@V matmul).
"""

import os
import sys
import math

sys.path.insert(0, "/opt/trn_rl_repo")

import numpy as np
import ml_dtypes

import concourse.bass as bass
import concourse.mybir as mybir
from concourse import tile, bacc, masks
from concourse.bass_utils import run_bass_kernel_spmd

# ---------------- problem constants ----------------
DIM = 4096
N_HEADS = 32
N_KV_HEADS = 8
HEAD_DIM = 128
NCORE = 8
HPC = N_HEADS // NCORE            # 4 query heads per core
QF = HPC * HEAD_DIM               # 512 features per core
BSZ = (16, 16)
SP = (2048, 1024)                 # start_pos per group
TOT_B = 32
NFULL = (SP[0] // 128, SP[1] // 128)   # full 128-pos chunks per group: 16, 8
KCH = DIM // 128                  # 32 contraction chunks

USE_F32 = bool(int(os.environ.get("BASS_ATTN_F32", "0")))
DT = mybir.dt.float32 if USE_F32 else mybir.dt.bfloat16
NPDT = np.float32 if USE_F32 else ml_dtypes.bfloat16
SPT = 1 if USE_F32 else 2          # samples per KV tile (f32 tiles are 2x bytes)
WQ_BUFS = int(os.environ.get("BASS_WQ_BUFS", "4"))

f32 = mybir.dt.float32


def _build_nc():
    nc = bacc.Bacc(trn_type="TRN2", num_devices=NCORE, enable_asserts=True)

    # ---- I/O ----
    xh = nc.dram_tensor("xh", [128, KCH, TOT_B], DT, kind="ExternalInput")
    wqkv = nc.dram_tensor("wqkv", [128, KCH, QF + 2 * HEAD_DIM], DT, kind="ExternalInput")
    # wo in [local_c, f] layout: wo_cf[p, h, f] = wo[f, 512*r + h*128 + p]
    wo = nc.dram_tensor("wo", [128, HPC, DIM], DT, kind="ExternalInput")
    kt0 = nc.dram_tensor("kt0", [BSZ[0], 128, SP[0]], DT, kind="ExternalInput")
    kt1 = nc.dram_tensor("kt1", [BSZ[1], 128, SP[1]], DT, kind="ExternalInput")
    vp0 = nc.dram_tensor("vp0", [BSZ[0], 128, NFULL[0], 129], DT, kind="ExternalInput")
    vp1 = nc.dram_tensor("vp1", [BSZ[1], 128, NFULL[1], 129], DT, kind="ExternalInput")
    ropec = nc.dram_tensor("ropec", [128, TOT_B], f32, kind="ExternalInput")
    ropes = nc.dram_tensor("ropes", [128, TOT_B], f32, kind="ExternalInput")
    # yT: rows = this core's 512 output features (f = 512*r + row), cols = samples
    # (collectives may not write IO tensors - the BIR verifier rejects it -
    # so ReduceScatter lands in rs_out and a small DMA ships it to y)
    y = nc.dram_tensor("y", [QF, TOT_B], f32, kind="ExternalOutput")

    WQKV_W = QF + 2 * HEAD_DIM  # 768
    SWAP_MASK = [i ^ 1 for i in range(32)]

    with tile.TileContext(nc) as tc:
        with tc.tile_pool(name="cpool", bufs=1) as cpool, \
             tc.tile_pool(name="wpool", bufs=2) as wpool, \
             tc.tile_pool(name="kvpool", bufs=int(os.environ.get("BASS_KV_BUFS", "3"))) as kvpool, \
             tc.tile_pool(name="apool", bufs=3) as apool, \
             tc.tile_pool(name="ps_t", bufs=3, space="PSUM") as ps_t, \
             tc.tile_pool(name="dpool", bufs=1, space="DRAM") as dpool:

            # ---------- constants ----------
            ident = cpool.tile([128, 128], f32)
            masks.make_identity(nc, ident[:])
            identdt = cpool.tile([TOT_B, TOT_B], DT)
            masks.make_identity(nc, identdt[:])

            # x + wqkv go at the head of the SP ring (same ring as the KV
            # stream) so the QKV critical chain gets full DMA bandwidth
            # before the bulk KV traffic.
            x_sb = cpool.tile([128, KCH * TOT_B], DT)
            nc.sync.dma_start(x_sb[:].rearrange("p (c b) -> p c b", c=KCH), xh[:])
            ropec_sb = cpool.tile([128, TOT_B], f32)
            nc.scalar.dma_start(ropec_sb[:], ropec[:])
            ropes_sb = cpool.tile([128, TOT_B], f32)
            nc.scalar.dma_start(ropes_sb[:], ropes[:])

            # ---------- phase A: QKV projection ----------
            with tc.tile_pool(name="ps_a", bufs=1, space="PSUM") as ps_a:
                qkv_ps = ps_a.tile([TOT_B, WQKV_W], f32)
                for P in range(4):
                    wq_t = wpool.tile([128, 8 * WQKV_W], DT, tag="wq", bufs=WQ_BUFS)
                    nc.sync.dma_start(
                        wq_t[:].rearrange("p (c j) -> p c j", c=8),
                        wqkv[:, 8 * P:8 * P + 8, :],
                    )
                    for ci in range(8):
                        c = 8 * P + ci
                        lhs = x_sb[:, TOT_B * c:TOT_B * (c + 1)]
                        rhs = wq_t[:, WQKV_W * ci:WQKV_W * (ci + 1)]
                        nc.tensor.matmul(qkv_ps[:, 0:512], lhs, rhs[:, 0:512],
                                         start=(c == 0), stop=(c == KCH - 1))
                        nc.tensor.matmul(qkv_ps[:, 512:768], lhs, rhs[:, 512:768],
                                         start=(c == 0), stop=(c == KCH - 1))

                qkv_sb = cpool.tile([TOT_B, WQKV_W], f32)
                nc.scalar.copy(qkv_sb[:], qkv_ps[:])

            # wo weights prefetch tile. Issued on the SP ring between wqkv and
            # the KV stream: PE's in-order SEQ hits the first group's
            # partial-wo matmuls right after that group's attention, so wo
            # must be resident by ~55us or PE stalls and the second group's
            # whole pipeline backs up. (Not at the very top: a single 4.2MB
            # DMACopy would hold the shared DMA pool ahead of wqkv and delay
            # the QKV projection that gates all attention.)
            wo_all = wpool.tile([128, KCH * QF], DT, tag="wo", bufs=1)
            nc.sync.dma_start(
                wo_all[:].rearrange("p (c j) -> p c j", c=HPC), wo[:])

            # new-position V (plus ones column for the softmax denominator)
            vnew = cpool.tile([TOT_B, 129], DT)
            nc.vector.tensor_copy(vnew[:, 0:HEAD_DIM], qkv_sb[:, 640:768])
            nc.vector.memset(vnew[:, 128:129], 1.0)

            # ---------- transpose q heads + k, apply RoPE ----------
            qT4 = cpool.tile([128, HPC * TOT_B], DT)   # col = b*4 + h
            kTn = cpool.tile([128, TOT_B], DT)         # col = b
            for h in range(HPC + 1):                   # 4 q heads then k
                tp = ps_t.tile([128, TOT_B], f32, tag="tp")
                nc.tensor.transpose(tp[:], qkv_sb[:, 128 * h:128 * (h + 1)],
                                    ident[0:TOT_B, 0:TOT_B])
                t_sb = apool.tile([128, TOT_B], f32, tag="tr")
                nc.vector.tensor_copy(t_sb[:], tp[:])
                sw = apool.tile([128, TOT_B], f32, tag="sw")
                nc.vector.stream_shuffle(sw[:], t_sb[:], SWAP_MASK)
                t1 = apool.tile([128, TOT_B], f32, tag="t1")
                nc.vector.tensor_mul(t1[:], t_sb[:], ropec_sb[:])
                nc.vector.tensor_mul(sw[:], sw[:], ropes_sb[:])
                if h < HPC:
                    dest = qT4[:, h::HPC]
                else:
                    dest = kTn[:]
                nc.vector.tensor_add(dest, t1[:], sw[:])

            # ---------- phase B: attention over the KV cache ----------
            attnT = cpool.tile([128, HPC * TOT_B], DT)  # col = h*32 + b
            kts = (kt0, kt1)
            vps = (vp0, vp1)
            # Output projection is reduction-parallel: each core computes its
            # partial wo product (transposed, [4096, 16] per sample group) and
            # a ReduceScatter(add) sums across cores, leaving each core its
            # own 512 output-feature rows. Group 1 (1024-pos, half the KV
            # bytes) goes FIRST so its collective fires ~54us in, fully
            # overlapped by group 0's KV stream; only group 0's collective
            # sits in the tail. COLLECTIVE_CORES is exclusive, so two
            # back-to-back collectives at the tail would serialize 2x15.8us.
            rs_in = [dpool.tile([DIM, 16], f32, name=f"rs_in{g}") for g in range(2)]
            rs_out = [dpool.tile([QF, 16], f32, name=f"rs_out{g}")
                      for g in range(2)]
            with tc.tile_pool(name="ps_b", bufs=2, space="PSUM") as ps_b:
                for gi, g in enumerate((1, 0)):
                    npos = SP[g]
                    nf = NFULL[g]
                    ncol = 4 * nf
                    vw = 129 * nf
                    # Small KV slices keep the shared DMA resource's FIFO
                    # shallow: rs_in/y writes and the collectives' inputs
                    # would otherwise queue behind multiple megabyte-sized KV
                    # reads (observed +16us on the first collective). Taper
                    # the tail of the LAST group so the serial per-sample
                    # attention chain after the final DMA byte is short.
                    if SPT == 1:
                        blocks = [1] * BSZ[g]
                    elif gi == 1:
                        blocks = [2] * 7 + [1, 1]
                    else:
                        blocks = [2] * (BSZ[g] // 2)
                    s_off = 0
                    for blk in blocks:
                        ktile = kvpool.tile([128, SPT * SP[0]], DT, tag="kt")
                        nc.sync.dma_start(
                            ktile[:, 0:blk * npos].rearrange("p (s n) -> p s n", s=blk),
                            kts[g][s_off:s_off + blk].rearrange("s p n -> p s n"),
                        )
                        vtile = kvpool.tile([128, SPT * 129 * NFULL[0]], DT, tag="vt")
                        nc.sync.dma_start(
                            vtile[:, 0:blk * vw].rearrange("p (s c d) -> p s c d", s=blk, c=nf),
                            vps[g][s_off:s_off + blk].rearrange("s p c d -> p s c d"),
                        )
                        for j in range(blk):
                            b = 16 * g + s_off + j
                            ks = ktile[:, j * npos:(j + 1) * npos]
                            vs = vtile[:, j * vw:(j + 1) * vw]
                            q_b = qT4[:, HPC * b:HPC * (b + 1)]

                            sc_ps = ps_b.tile([128, 68], f32, tag="sc")
                            for c in range(nf):
                                nc.tensor.matmul(sc_ps[:, 4 * c:4 * c + 4],
                                                 ks[:, 128 * c:128 * (c + 1)], q_b,
                                                 start=True, stop=True)
                            nc.tensor.matmul(sc_ps[0:1, ncol:ncol + 4],
                                             kTn[:, b:b + 1], q_b,
                                             start=True, stop=True)

                            # one exp over the scores plus the new-position
                            # row (cols ncol:ncol+4 rows 1.. hold stale psum
                            # floats that are exp'd but never read)
                            pr = apool.tile([128, 68], DT, tag="pr")
                            nc.scalar.activation(pr[:, 0:ncol + 4],
                                                 sc_ps[:, 0:ncol + 4],
                                                 mybir.ActivationFunctionType.Exp)

                            # select row b of vnew into partition 0 (psum), for the
                            # tail matmul rhs (moving operand must be partition-0 based)
                            vrow_ps = ps_b.tile([1, 129], f32, tag="vr", bufs=1)
                            nc.tensor.matmul(vrow_ps[:], identdt[:, b:b + 1], vnew[:],
                                             start=True, stop=True)
                            vrow = apool.tile([1, 129], DT, tag="vrow")
                            nc.vector.tensor_copy(vrow[:], vrow_ps[:])

                            o_ps = ps_b.tile([HPC, 129], f32, tag="o")
                            for c in range(nf):
                                nc.tensor.matmul(o_ps[:], pr[:, 4 * c:4 * c + 4],
                                                 vs[:, 129 * c:129 * (c + 1)],
                                                 start=(c == 0), stop=False)
                            nc.tensor.matmul(o_ps[:], pr[0:1, ncol:ncol + 4],
                                             vrow[:], start=False, stop=True)

                            rec = apool.tile([HPC, 1], f32, tag="rec")
                            nc.vector.reciprocal(rec[:], o_ps[:, 128:129])
                            at = apool.tile([HPC, HEAD_DIM], f32, tag="at")
                            nc.vector.tensor_scalar_mul(at[:], o_ps[:, 0:HEAD_DIM], rec[:])

                            tp2 = ps_t.tile([128, TOT_B], f32, tag="tp")
                            nc.tensor.transpose(tp2[:, 0:HPC], at[:], ident[0:HPC, 0:HPC])
                            nc.vector.tensor_copy(attnT[:, b::TOT_B], tp2[:, 0:HPC])
                        s_off += blk

                    # this group's samples are done: partial wo product
                    # partialT[f, b] = sum_c wo[f, c] * attn[b, c]  (c = own features)
                    pT_sb = apool.tile([128, 32 * 16], f32, tag="pt", bufs=2)
                    for fq in range(4):          # 8 fb blocks per PSUM bank
                        pt_ps = ps_t.tile([128, 128], f32, tag="tp")
                        for fi in range(8):
                            fb = 8 * fq + fi
                            for h in range(HPC):
                                nc.tensor.matmul(
                                    pt_ps[:, 16 * fi:16 * (fi + 1)],
                                    wo_all[:, h * DIM + 128 * fb:h * DIM + 128 * (fb + 1)],
                                    attnT[:, TOT_B * h + 16 * g:TOT_B * h + 16 * (g + 1)],
                                    start=(h == 0), stop=(h == HPC - 1))
                        nc.vector.tensor_copy(pT_sb[:, 128 * fq:128 * (fq + 1)], pt_ps[:])
                        # rs_in row order is permuted within each rank's
                        # 512-row block (row = 512r + 4p + fb%4) so DRAM
                        # writes are 256B contiguous runs instead of 64B;
                        # host un-permutes (the global mapping rank=fb//4,
                        # row-in-rank=4p+fb%4 is chunk-size invariant).
                        # Shipped in two rank-halves; the tail group's go on
                        # the idle ACT HWDGE ring (faster first-byte than
                        # SWDGE), the first group's on gpsimd mid-stream.
                        if fq == 1 or fq == 3:
                            half = fq // 2
                            dma_eng = nc.scalar if gi == 1 else nc.gpsimd
                            dma_eng.dma_start(
                                rs_in[g][2048 * half:2048 * (half + 1)].rearrange(
                                    "(r p four) b -> p r (four b)", r=NCORE // 2, four=HPC),
                                pT_sb[:, 256 * half:256 * (half + 1)].rearrange(
                                    "p (r four b) -> p r (four b)", r=NCORE // 2, four=HPC),
                            )
                    nc.gpsimd.collective_compute(
                        "ReduceScatter", mybir.AluOpType.add,
                        replica_groups=[list(range(NCORE))],
                        ins=[rs_in[g].opt()], outs=[rs_out[g].opt()],
                    )

                # y writebacks go on the SP ring AFTER both groups' KV
                # dma_starts are queued: a y write waits on its collective,
                # and anything queued behind it on the same in-order ring
                # would stall until the collective completes.
                for g in (1, 0):
                    nc.sync.dma_start(y[:, 16 * g:16 * (g + 1)], rs_out[g][:])

    nc.finalize()
    return nc


_NC_CACHE = None


def _get_nc():
    global _NC_CACHE
    if _NC_CACHE is None:
        _NC_CACHE = _build_nc()
    return _NC_CACHE


def _prep_inputs(inputs):
    """Shard + lay out the full inputs for the 8 cores."""
    x = np.asarray(inputs["x"], np.float32)
    wq = np.asarray(inputs["wq"], np.float32)
    wk = np.asarray(inputs["wk"], np.float32)
    wv = np.asarray(inputs["wv"], np.float32)
    wo = np.asarray(inputs["wo"], np.float32)
    fc = np.asarray(inputs["freqs_cos"], np.float32)
    fs = np.asarray(inputs["freqs_sin"], np.float32)
    caches = (
        (np.asarray(inputs["cache_k0"], np.float32), np.asarray(inputs["cache_v0"], np.float32)),
        (np.asarray(inputs["cache_k1"], np.float32), np.asarray(inputs["cache_v1"], np.float32)),
    )

    x_flat = x.reshape(TOT_B, DIM)
    xh = np.ascontiguousarray(
        x_flat.T.reshape(KCH, 128, TOT_B).transpose(1, 0, 2)
    ).astype(NPDT)

    # RoPE tables: per-column position (2048 for tokens 0-15, 1024 for 16-31)
    C = np.empty((128, TOT_B), np.float32)
    S = np.empty((128, TOT_B), np.float32)
    for g in range(2):
        cos = fc[SP[g]]
        sin = fs[SP[g]]
        cols = slice(16 * g, 16 * (g + 1))
        C[0::2, cols] = cos[:, None]
        C[1::2, cols] = cos[:, None]
        S[0::2, cols] = -sin[:, None]
        S[1::2, cols] = sin[:, None]

    scale = 1.0 / math.sqrt(HEAD_DIM)

    def _prep_core(r):
        w_q = wq[QF * r:QF * (r + 1)] * scale
        w_k = wk[HEAD_DIM * r:HEAD_DIM * (r + 1)]
        w_v = wv[HEAD_DIM * r:HEAD_DIM * (r + 1)]
        wqkvT = np.concatenate([w_q, w_k, w_v], axis=0).T  # [4096, 768]
        wqkv_hp = np.ascontiguousarray(
            wqkvT.reshape(KCH, 128, 768).transpose(1, 0, 2)
        ).astype(NPDT)

        # wo_cf[local_c, f] = wo[f, 512r + local_c]  -> [128, HPC, 4096]
        wo_cf = wo[:, QF * r:QF * (r + 1)].T  # [512, 4096]
        wo_hp = np.ascontiguousarray(
            wo_cf.reshape(HPC, 128, DIM).transpose(1, 0, 2)
        ).astype(NPDT)

        m = {"xh": xh, "wqkv": wqkv_hp, "wo": wo_hp,
             "ropec": C, "ropes": S}
        for g in range(2):
            ck, cv = caches[g]
            npos = SP[g]
            nf = NFULL[g]
            # cast to the wire dtype first, then do the layout copy at half width
            kslab = ck[:, :npos, r, :].astype(NPDT)       # [16, npos, 128]
            kt = np.ascontiguousarray(kslab.transpose(0, 2, 1))  # [16, 128, npos]
            vslab = cv[:, :npos, r, :].astype(NPDT).reshape(BSZ[g], nf, 128, HEAD_DIM)
            vp = np.empty((BSZ[g], 128, nf, 129), NPDT)
            vp[:, :, :, HEAD_DIM] = NPDT(1.0)
            vp[:, :, :, :HEAD_DIM] = vslab.transpose(0, 2, 1, 3)
            m[f"kt{g}"] = kt
            m[f"vp{g}"] = vp
        return m

    from concurrent.futures import ThreadPoolExecutor
    with ThreadPoolExecutor(max_workers=NCORE) as ex:
        in_maps = list(ex.map(_prep_core, range(NCORE)))
    return in_maps


def _run(inputs, trace=False):
    nc = _get_nc()
    in_maps = _prep_inputs(inputs)
    res = run_bass_kernel_spmd(nc, in_maps, core_ids=list(range(NCORE)), trace=trace)
    # each core returns yT rows [512r : 512r+512] of the [4096, 32] output,
    # row-permuted within the block (row = 4p + fb%4 -> f_local = 128*(fb%4) + p)
    parts = []
    for r in range(NCORE):
        yr = res.results[r]["y"]  # [512, 32]
        parts.append(yr.reshape(128, HPC, TOT_B).transpose(1, 0, 2).reshape(QF, TOT_B))
    y_t = np.concatenate(parts, axis=0)
    out = np.ascontiguousarray(y_t.T).reshape(TOT_B, 1, DIM).astype(np.float32)
    return out, res


def kernel(**inputs):
    try:
        out, _ = _run(inputs, trace=False)
    except Exception:
        # transient NRT/axon hiccups have been observed to recover on retry
        out, _ = _run(inputs, trace=False)
    return out



# revision 41
# speedup vs baseline: 1.0427x; 1.0116x over previous
"""Trainium2 Bass kernel for GQA decode attention (nn_Attention_45844480917562).

Tensor-parallel over 8 NeuronCores: each core owns 4 query heads + 1 KV head
(wq/wk/wv column-sharded). The output projection is reduction-parallel: each
core computes its partial wo product transposed and a per-sample-group
ReduceScatter(add) leaves each core its own 512 output-feature rows; the host
only concatenates/transposes.

Compute dtype is bf16 (fp32 PSUM accumulation, fp32 softmax denominator /
division); BASS_ATTN_F32=1 switches to full fp32 at ~2x the HBM traffic.

Self-contained: hardcodes all shapes; host-side prep reshapes/transposes the
full inputs into per-core DMA-friendly layouts (K cache transposed to
[head_dim, pos], V cache chunk-major with a fused ones-column that yields the
softmax denominator for free in the P@V matmul).
"""

import os
import sys
import math

sys.path.insert(0, "/opt/trn_rl_repo")

import numpy as np
import ml_dtypes

import concourse.bass as bass
import concourse.mybir as mybir
from concourse import tile, bacc, masks
from concourse.bass_utils import run_bass_kernel_spmd

# ---------------- problem constants ----------------
DIM = 4096
N_HEADS = 32
N_KV_HEADS = 8
HEAD_DIM = 128
NCORE = 8
HPC = N_HEADS // NCORE            # 4 query heads per core
QF = HPC * HEAD_DIM               # 512 features per core
BSZ = (16, 16)
SP = (2048, 1024)                 # start_pos per group
TOT_B = 32
NFULL = (SP[0] // 128, SP[1] // 128)   # full 128-pos chunks per group: 16, 8
KCH = DIM // 128                  # 32 contraction chunks

USE_F32 = bool(int(os.environ.get("BASS_ATTN_F32", "0")))
DT = mybir.dt.float32 if USE_F32 else mybir.dt.bfloat16
NPDT = np.float32 if USE_F32 else ml_dtypes.bfloat16
SPT = 1 if USE_F32 else 2          # samples per KV tile (f32 tiles are 2x bytes)
WQ_BUFS = int(os.environ.get("BASS_WQ_BUFS", "4"))

f32 = mybir.dt.float32


def _build_nc():
    nc = bacc.Bacc(trn_type="TRN2", num_devices=NCORE, enable_asserts=True)

    # ---- I/O ----
    xh = nc.dram_tensor("xh", [128, KCH, TOT_B], DT, kind="ExternalInput")
    wqkv = nc.dram_tensor("wqkv", [128, KCH, QF + 2 * HEAD_DIM], DT, kind="ExternalInput")
    # wo in [local_c, f] layout: wo_cf[p, h, f] = wo[f, 512*r + h*128 + p]
    wo = nc.dram_tensor("wo", [128, HPC, DIM], DT, kind="ExternalInput")
    kt0 = nc.dram_tensor("kt0", [BSZ[0], 128, SP[0]], DT, kind="ExternalInput")
    kt1 = nc.dram_tensor("kt1", [BSZ[1], 128, SP[1]], DT, kind="ExternalInput")
    vp0 = nc.dram_tensor("vp0", [BSZ[0], 128, NFULL[0], 129], DT, kind="ExternalInput")
    vp1 = nc.dram_tensor("vp1", [BSZ[1], 128, NFULL[1], 129], DT, kind="ExternalInput")
    ropec = nc.dram_tensor("ropec", [128, TOT_B], f32, kind="ExternalInput")
    ropes = nc.dram_tensor("ropes", [128, TOT_B], f32, kind="ExternalInput")
    # yT: rows = this core's 512 output features (f = 512*r + row), cols = samples
    # (collectives may not write IO tensors - the BIR verifier rejects it -
    # so ReduceScatter lands in rs_out and a small DMA ships it to y).
    # The whole reduce path runs in the compute dtype (bf16): halves the
    # collective's charged bytes and the rs_in/y DMA traffic; host casts back.
    y = nc.dram_tensor("y", [QF, TOT_B], DT, kind="ExternalOutput")

    WQKV_W = QF + 2 * HEAD_DIM  # 768
    SWAP_MASK = [i ^ 1 for i in range(32)]

    with tile.TileContext(nc) as tc:
        with tc.tile_pool(name="cpool", bufs=1) as cpool, \
             tc.tile_pool(name="wpool", bufs=2) as wpool, \
             tc.tile_pool(name="kvpool", bufs=int(os.environ.get("BASS_KV_BUFS", "3"))) as kvpool, \
             tc.tile_pool(name="apool", bufs=3) as apool, \
             tc.tile_pool(name="ps_t", bufs=3, space="PSUM") as ps_t, \
             tc.tile_pool(name="dpool", bufs=1, space="DRAM") as dpool:

            # ---------- constants ----------
            ident = cpool.tile([128, 128], f32)
            masks.make_identity(nc, ident[:])
            identdt = cpool.tile([TOT_B, TOT_B], DT)
            masks.make_identity(nc, identdt[:])

            # x + wqkv go at the head of the stream so the QKV critical chain
            # gets full DMA bandwidth before the bulk KV traffic. Alternating
            # wqkv chunks between the SP and ACT HWDGE rings pipelines their
            # dispatch/desc-gen latency; rope tables (needed only at ~19us)
            # are issued after chunk 0 to keep them off the critical ramp.
            x_sb = cpool.tile([128, KCH * TOT_B], DT)
            nc.sync.dma_start(x_sb[:].rearrange("p (c b) -> p c b", c=KCH), xh[:])
            ropec_sb = cpool.tile([128, TOT_B], f32)
            ropes_sb = cpool.tile([128, TOT_B], f32)

            # ---------- phase A: QKV projection ----------
            with tc.tile_pool(name="ps_a", bufs=1, space="PSUM") as ps_a:
                qkv_ps = ps_a.tile([TOT_B, WQKV_W], f32)
                for P in range(4):
                    wq_t = wpool.tile([128, 8 * WQKV_W], DT, tag="wq", bufs=WQ_BUFS)
                    nc.sync.dma_start(
                        wq_t[:].rearrange("p (c j) -> p c j", c=8),
                        wqkv[:, 8 * P:8 * P + 8, :],
                    )
                    if P == 0:
                        nc.scalar.dma_start(ropec_sb[:], ropec[:])
                        nc.scalar.dma_start(ropes_sb[:], ropes[:])
                    for ci in range(8):
                        c = 8 * P + ci
                        lhs = x_sb[:, TOT_B * c:TOT_B * (c + 1)]
                        rhs = wq_t[:, WQKV_W * ci:WQKV_W * (ci + 1)]
                        nc.tensor.matmul(qkv_ps[:, 0:512], lhs, rhs[:, 0:512],
                                         start=(c == 0), stop=(c == KCH - 1))
                        nc.tensor.matmul(qkv_ps[:, 512:768], lhs, rhs[:, 512:768],
                                         start=(c == 0), stop=(c == KCH - 1))

                qkv_sb = cpool.tile([TOT_B, WQKV_W], f32)
                nc.scalar.copy(qkv_sb[:], qkv_ps[:])

            # wo weights prefetch tile. Issued on the SP ring between wqkv and
            # the KV stream: PE's in-order SEQ hits the first group's
            # partial-wo matmuls right after that group's attention, so wo
            # must be resident by ~55us or PE stalls and the second group's
            # whole pipeline backs up. (Not at the very top: a single 4.2MB
            # DMACopy would hold the shared DMA pool ahead of wqkv and delay
            # the QKV projection that gates all attention.)
            wo_all = wpool.tile([128, KCH * QF], DT, tag="wo", bufs=1)
            nc.sync.dma_start(
                wo_all[:].rearrange("p (c j) -> p c j", c=HPC), wo[:])

            # new-position V (plus ones column for the softmax denominator)
            vnew = cpool.tile([TOT_B, 129], DT)
            nc.vector.tensor_copy(vnew[:, 0:HEAD_DIM], qkv_sb[:, 640:768])
            nc.vector.memset(vnew[:, 128:129], 1.0)

            # ---------- transpose q heads + k, apply RoPE ----------
            qT4 = cpool.tile([128, HPC * TOT_B], DT)   # col = b*4 + h
            kTn = cpool.tile([128, TOT_B], DT)         # col = b
            for h in range(HPC + 1):                   # 4 q heads then k
                tp = ps_t.tile([128, TOT_B], f32, tag="tp")
                nc.tensor.transpose(tp[:], qkv_sb[:, 128 * h:128 * (h + 1)],
                                    ident[0:TOT_B, 0:TOT_B])
                t_sb = apool.tile([128, TOT_B], f32, tag="tr")
                nc.vector.tensor_copy(t_sb[:], tp[:])
                sw = apool.tile([128, TOT_B], f32, tag="sw")
                nc.vector.stream_shuffle(sw[:], t_sb[:], SWAP_MASK)
                t1 = apool.tile([128, TOT_B], f32, tag="t1")
                nc.vector.tensor_mul(t1[:], t_sb[:], ropec_sb[:])
                nc.vector.tensor_mul(sw[:], sw[:], ropes_sb[:])
                if h < HPC:
                    dest = qT4[:, h::HPC]
                else:
                    dest = kTn[:]
                nc.vector.tensor_add(dest, t1[:], sw[:])

            # ---------- phase B: attention over the KV cache ----------
            attnT = cpool.tile([128, HPC * TOT_B], DT)  # col = h*32 + b
            kts = (kt0, kt1)
            vps = (vp0, vp1)
            # Output projection is reduction-parallel: each core computes its
            # partial wo product (transposed, [4096, 16] per sample group) and
            # a ReduceScatter(add) sums across cores, leaving each core its
            # own 512 output-feature rows. Group 1 (1024-pos, half the KV
            # bytes) goes FIRST so its collective fires ~54us in, fully
            # overlapped by group 0's KV stream; only group 0's collective
            # sits in the tail. COLLECTIVE_CORES is exclusive, so two
            # back-to-back collectives at the tail would serialize 2x15.8us.
            rs_in = [dpool.tile([DIM, 16], DT, name=f"rs_in{g}") for g in range(2)]
            rs_out = [dpool.tile([QF, 16], DT, name=f"rs_out{g}")
                      for g in range(2)]
            with tc.tile_pool(name="ps_b", bufs=2, space="PSUM") as ps_b:
                for gi, g in enumerate((1, 0)):
                    npos = SP[g]
                    nf = NFULL[g]
                    ncol = 4 * nf
                    vw = 129 * nf
                    # Small KV slices keep the shared DMA resource's FIFO
                    # shallow: rs_in/y writes and the collectives' inputs
                    # would otherwise queue behind multiple megabyte-sized KV
                    # reads (observed +16us on the first collective). Taper
                    # the tail of the LAST group so the serial per-sample
                    # attention chain after the final DMA byte is short.
                    if SPT == 1:
                        blocks = [1] * BSZ[g]
                    elif gi == 1:
                        blocks = [2] * 7 + [1, 1]
                    else:
                        blocks = [2] * (BSZ[g] // 2)
                    s_off = 0
                    for blk in blocks:
                        ktile = kvpool.tile([128, SPT * SP[0]], DT, tag="kt")
                        nc.sync.dma_start(
                            ktile[:, 0:blk * npos].rearrange("p (s n) -> p s n", s=blk),
                            kts[g][s_off:s_off + blk].rearrange("s p n -> p s n"),
                        )
                        vtile = kvpool.tile([128, SPT * 129 * NFULL[0]], DT, tag="vt")
                        nc.sync.dma_start(
                            vtile[:, 0:blk * vw].rearrange("p (s c d) -> p s c d", s=blk, c=nf),
                            vps[g][s_off:s_off + blk].rearrange("s p c d -> p s c d"),
                        )
                        for j in range(blk):
                            b = 16 * g + s_off + j
                            ks = ktile[:, j * npos:(j + 1) * npos]
                            vs = vtile[:, j * vw:(j + 1) * vw]
                            q_b = qT4[:, HPC * b:HPC * (b + 1)]

                            sc_ps = ps_b.tile([128, 68], f32, tag="sc")
                            for c in range(nf):
                                nc.tensor.matmul(sc_ps[:, 4 * c:4 * c + 4],
                                                 ks[:, 128 * c:128 * (c + 1)], q_b,
                                                 start=True, stop=True)
                            nc.tensor.matmul(sc_ps[0:1, ncol:ncol + 4],
                                             kTn[:, b:b + 1], q_b,
                                             start=True, stop=True)

                            # one exp over the scores plus the new-position
                            # row (cols ncol:ncol+4 rows 1.. hold stale psum
                            # floats that are exp'd but never read)
                            pr = apool.tile([128, 68], DT, tag="pr")
                            nc.scalar.activation(pr[:, 0:ncol + 4],
                                                 sc_ps[:, 0:ncol + 4],
                                                 mybir.ActivationFunctionType.Exp)

                            # select row b of vnew into partition 0 (psum), for the
                            # tail matmul rhs (moving operand must be partition-0 based)
                            vrow_ps = ps_b.tile([1, 129], f32, tag="vr", bufs=1)
                            nc.tensor.matmul(vrow_ps[:], identdt[:, b:b + 1], vnew[:],
                                             start=True, stop=True)
                            vrow = apool.tile([1, 129], DT, tag="vrow")
                            nc.vector.tensor_copy(vrow[:], vrow_ps[:])

                            o_ps = ps_b.tile([HPC, 129], f32, tag="o")
                            for c in range(nf):
                                nc.tensor.matmul(o_ps[:], pr[:, 4 * c:4 * c + 4],
                                                 vs[:, 129 * c:129 * (c + 1)],
                                                 start=(c == 0), stop=False)
                            nc.tensor.matmul(o_ps[:], pr[0:1, ncol:ncol + 4],
                                             vrow[:], start=False, stop=True)

                            rec = apool.tile([HPC, 1], f32, tag="rec")
                            nc.vector.reciprocal(rec[:], o_ps[:, 128:129])
                            at = apool.tile([HPC, HEAD_DIM], f32, tag="at")
                            nc.vector.tensor_scalar_mul(at[:], o_ps[:, 0:HEAD_DIM], rec[:])

                            tp2 = ps_t.tile([128, TOT_B], f32, tag="tp")
                            nc.tensor.transpose(tp2[:, 0:HPC], at[:], ident[0:HPC, 0:HPC])
                            nc.vector.tensor_copy(attnT[:, b::TOT_B], tp2[:, 0:HPC])
                        s_off += blk

                    # this group's samples are done: partial wo product
                    # partialT[f, b] = sum_c wo[f, c] * attn[b, c]  (c = own features)
                    pT_sb = apool.tile([128, 32 * 16], DT, tag="pt", bufs=2)
                    for fq in range(4):          # 8 fb blocks per PSUM bank
                        pt_ps = ps_t.tile([128, 128], f32, tag="tp")
                        for fi in range(8):
                            fb = 8 * fq + fi
                            for h in range(HPC):
                                nc.tensor.matmul(
                                    pt_ps[:, 16 * fi:16 * (fi + 1)],
                                    wo_all[:, h * DIM + 128 * fb:h * DIM + 128 * (fb + 1)],
                                    attnT[:, TOT_B * h + 16 * g:TOT_B * h + 16 * (g + 1)],
                                    start=(h == 0), stop=(h == HPC - 1))
                        nc.vector.tensor_copy(pT_sb[:, 128 * fq:128 * (fq + 1)], pt_ps[:])
                        # rs_in row order is permuted within each rank's
                        # 512-row block (row = 512r + 4p + fb%4) so DRAM
                        # writes are 256B contiguous runs instead of 64B;
                        # host un-permutes (the global mapping rank=fb//4,
                        # row-in-rank=4p+fb%4 is chunk-size invariant).
                        # Shipped in two rank-halves; the tail group's go on
                        # the idle ACT HWDGE ring (faster first-byte than
                        # SWDGE), the first group's on gpsimd mid-stream.
                        if fq == 1 or fq == 3:
                            half = fq // 2
                            dma_eng = nc.scalar if gi == 1 else nc.gpsimd
                            dma_eng.dma_start(
                                rs_in[g][2048 * half:2048 * (half + 1)].rearrange(
                                    "(r p four) b -> p r (four b)", r=NCORE // 2, four=HPC),
                                pT_sb[:, 256 * half:256 * (half + 1)].rearrange(
                                    "p (r four b) -> p r (four b)", r=NCORE // 2, four=HPC),
                            )
                    nc.gpsimd.collective_compute(
                        "ReduceScatter", mybir.AluOpType.add,
                        replica_groups=[list(range(NCORE))],
                        ins=[rs_in[g].opt()], outs=[rs_out[g].opt()],
                    )

                # y writebacks go on the SP ring AFTER both groups' KV
                # dma_starts are queued: a y write waits on its collective,
                # and anything queued behind it on the same in-order ring
                # would stall until the collective completes.
                for g in (1, 0):
                    nc.sync.dma_start(y[:, 16 * g:16 * (g + 1)], rs_out[g][:])

    nc.finalize()
    return nc


_NC_CACHE = None


def _get_nc():
    global _NC_CACHE
    if _NC_CACHE is None:
        _NC_CACHE = _build_nc()
    return _NC_CACHE


def _prep_inputs(inputs):
    """Shard + lay out the full inputs for the 8 cores."""
    x = np.asarray(inputs["x"], np.float32)
    wq = np.asarray(inputs["wq"], np.float32)
    wk = np.asarray(inputs["wk"], np.float32)
    wv = np.asarray(inputs["wv"], np.float32)
    wo = np.asarray(inputs["wo"], np.float32)
    fc = np.asarray(inputs["freqs_cos"], np.float32)
    fs = np.asarray(inputs["freqs_sin"], np.float32)
    caches = (
        (np.asarray(inputs["cache_k0"], np.float32), np.asarray(inputs["cache_v0"], np.float32)),
        (np.asarray(inputs["cache_k1"], np.float32), np.asarray(inputs["cache_v1"], np.float32)),
    )

    x_flat = x.reshape(TOT_B, DIM)
    xh = np.ascontiguousarray(
        x_flat.T.reshape(KCH, 128, TOT_B).transpose(1, 0, 2)
    ).astype(NPDT)

    # RoPE tables: per-column position (2048 for tokens 0-15, 1024 for 16-31)
    C = np.empty((128, TOT_B), np.float32)
    S = np.empty((128, TOT_B), np.float32)
    for g in range(2):
        cos = fc[SP[g]]
        sin = fs[SP[g]]
        cols = slice(16 * g, 16 * (g + 1))
        C[0::2, cols] = cos[:, None]
        C[1::2, cols] = cos[:, None]
        S[0::2, cols] = -sin[:, None]
        S[1::2, cols] = sin[:, None]

    scale = 1.0 / math.sqrt(HEAD_DIM)

    def _prep_core(r):
        w_q = wq[QF * r:QF * (r + 1)] * scale
        w_k = wk[HEAD_DIM * r:HEAD_DIM * (r + 1)]
        w_v = wv[HEAD_DIM * r:HEAD_DIM * (r + 1)]
        wqkvT = np.concatenate([w_q, w_k, w_v], axis=0).T  # [4096, 768]
        wqkv_hp = np.ascontiguousarray(
            wqkvT.reshape(KCH, 128, 768).transpose(1, 0, 2)
        ).astype(NPDT)

        # wo_cf[local_c, f] = wo[f, 512r + local_c]  -> [128, HPC, 4096]
        wo_cf = wo[:, QF * r:QF * (r + 1)].T  # [512, 4096]
        wo_hp = np.ascontiguousarray(
            wo_cf.reshape(HPC, 128, DIM).transpose(1, 0, 2)
        ).astype(NPDT)

        m = {"xh": xh, "wqkv": wqkv_hp, "wo": wo_hp,
             "ropec": C, "ropes": S}
        for g in range(2):
            ck, cv = caches[g]
            npos = SP[g]
            nf = NFULL[g]
            # cast to the wire dtype first, then do the layout copy at half width
            kslab = ck[:, :npos, r, :].astype(NPDT)       # [16, npos, 128]
            kt = np.ascontiguousarray(kslab.transpose(0, 2, 1))  # [16, 128, npos]
            vslab = cv[:, :npos, r, :].astype(NPDT).reshape(BSZ[g], nf, 128, HEAD_DIM)
            vp = np.empty((BSZ[g], 128, nf, 129), NPDT)
            vp[:, :, :, HEAD_DIM] = NPDT(1.0)
            vp[:, :, :, :HEAD_DIM] = vslab.transpose(0, 2, 1, 3)
            m[f"kt{g}"] = kt
            m[f"vp{g}"] = vp
        return m

    from concurrent.futures import ThreadPoolExecutor
    with ThreadPoolExecutor(max_workers=NCORE) as ex:
        in_maps = list(ex.map(_prep_core, range(NCORE)))
    return in_maps


def _run(inputs, trace=False):
    nc = _get_nc()
    in_maps = _prep_inputs(inputs)
    res = run_bass_kernel_spmd(nc, in_maps, core_ids=list(range(NCORE)), trace=trace)
    # each core returns yT rows [512r : 512r+512] of the [4096, 32] output,
    # row-permuted within the block (row = 4p + fb%4 -> f_local = 128*(fb%4) + p)
    parts = []
    for r in range(NCORE):
        yr = np.asarray(res.results[r]["y"], np.float32)  # [512, 32]
        parts.append(yr.reshape(128, HPC, TOT_B).transpose(1, 0, 2).reshape(QF, TOT_B))
    y_t = np.concatenate(parts, axis=0)
    out = np.ascontiguousarray(y_t.T).reshape(TOT_B, 1, DIM).astype(np.float32)
    return out, res


def kernel(**inputs):
    try:
        out, _ = _run(inputs, trace=False)
    except Exception:
        # transient NRT/axon hiccups have been observed to recover on retry
        out, _ = _run(inputs, trace=False)
    return out



# revision 45
# speedup vs baseline: 1.0455x; 1.0027x over previous
"""Trainium2 Bass kernel for GQA decode attention (nn_Attention_45844480917562).

Tensor-parallel over 8 NeuronCores: each core owns 4 query heads + 1 KV head
(wq/wk/wv column-sharded). The output projection is reduction-parallel: each
core computes its partial wo product transposed and a per-sample-group
ReduceScatter(add) leaves each core its own 512 output-feature rows; the host
only concatenates/transposes.

Compute dtype is bf16 (fp32 PSUM accumulation, fp32 softmax denominator /
division); BASS_ATTN_F32=1 switches to full fp32 at ~2x the HBM traffic.

Self-contained: hardcodes all shapes; host-side prep reshapes/transposes the
full inputs into per-core DMA-friendly layouts (K cache transposed to
[head_dim, pos], V cache chunk-major with a fused ones-column that yields the
softmax denominator for free in the P@V matmul).
"""

import os
import sys
import math

sys.path.insert(0, "/opt/trn_rl_repo")

import numpy as np
import ml_dtypes

import concourse.bass as bass
import concourse.mybir as mybir
from concourse import tile, bacc, masks
from concourse.bass_utils import run_bass_kernel_spmd

# ---------------- problem constants ----------------
DIM = 4096
N_HEADS = 32
N_KV_HEADS = 8
HEAD_DIM = 128
NCORE = 8
HPC = N_HEADS // NCORE            # 4 query heads per core
QF = HPC * HEAD_DIM               # 512 features per core
BSZ = (16, 16)
SP = (2048, 1024)                 # start_pos per group
TOT_B = 32
NFULL = (SP[0] // 128, SP[1] // 128)   # full 128-pos chunks per group: 16, 8
KCH = DIM // 128                  # 32 contraction chunks

USE_F32 = bool(int(os.environ.get("BASS_ATTN_F32", "0")))
# g0 (2048-pos) first: the tail group is then the 1024-pos one, whose
# last-sample attention chain after the final DMA byte is shorter.
GROUP_ORDER = (1, 0) if os.environ.get("BASS_G1_FIRST") else (0, 1)
DT = mybir.dt.float32 if USE_F32 else mybir.dt.bfloat16
NPDT = np.float32 if USE_F32 else ml_dtypes.bfloat16
SPT = 1 if USE_F32 else 2          # samples per KV tile (f32 tiles are 2x bytes)
WQ_BUFS = int(os.environ.get("BASS_WQ_BUFS", "4"))

f32 = mybir.dt.float32


def _build_nc():
    nc = bacc.Bacc(trn_type="TRN2", num_devices=NCORE, enable_asserts=True)

    # ---- I/O ----
    xh = nc.dram_tensor("xh", [128, KCH, TOT_B], DT, kind="ExternalInput")
    wqkv = nc.dram_tensor("wqkv", [128, KCH, QF + 2 * HEAD_DIM], DT, kind="ExternalInput")
    # wo in [local_c, f] layout: wo_cf[p, h, f] = wo[f, 512*r + h*128 + p]
    wo = nc.dram_tensor("wo", [128, HPC, DIM], DT, kind="ExternalInput")
    kt0 = nc.dram_tensor("kt0", [BSZ[0], 128, SP[0]], DT, kind="ExternalInput")
    kt1 = nc.dram_tensor("kt1", [BSZ[1], 128, SP[1]], DT, kind="ExternalInput")
    vp0 = nc.dram_tensor("vp0", [BSZ[0], 128, NFULL[0], 129], DT, kind="ExternalInput")
    vp1 = nc.dram_tensor("vp1", [BSZ[1], 128, NFULL[1], 129], DT, kind="ExternalInput")
    ropec = nc.dram_tensor("ropec", [128, TOT_B], f32, kind="ExternalInput")
    ropes = nc.dram_tensor("ropes", [128, TOT_B], f32, kind="ExternalInput")
    # yT: rows = this core's 512 output features (f = 512*r + row), cols = samples
    # (collectives may not write IO tensors - the BIR verifier rejects it -
    # so ReduceScatter lands in rs_out and a small DMA ships it to y).
    # The whole reduce path runs in the compute dtype (bf16): halves the
    # collective's charged bytes and the rs_in/y DMA traffic; host casts back.
    y = nc.dram_tensor("y", [QF, TOT_B], DT, kind="ExternalOutput")

    WQKV_W = QF + 2 * HEAD_DIM  # 768
    SWAP_MASK = [i ^ 1 for i in range(32)]

    with tile.TileContext(nc) as tc:
        with tc.tile_pool(name="cpool", bufs=1) as cpool, \
             tc.tile_pool(name="wpool", bufs=2) as wpool, \
             tc.tile_pool(name="kvpool", bufs=int(os.environ.get("BASS_KV_BUFS", "3"))) as kvpool, \
             tc.tile_pool(name="apool", bufs=3) as apool, \
             tc.tile_pool(name="ps_t", bufs=3, space="PSUM") as ps_t, \
             tc.tile_pool(name="dpool", bufs=1, space="DRAM") as dpool:

            # ---------- constants ----------
            ident = cpool.tile([128, 128], f32)
            masks.make_identity(nc, ident[:])
            identdt = cpool.tile([TOT_B, TOT_B], DT)
            masks.make_identity(nc, identdt[:])

            # x + wqkv go at the head of the stream so the QKV critical chain
            # gets full DMA bandwidth before the bulk KV traffic. Alternating
            # wqkv chunks between the SP and ACT HWDGE rings pipelines their
            # dispatch/desc-gen latency; rope tables (needed only at ~19us)
            # are issued after chunk 0 to keep them off the critical ramp.
            x_sb = cpool.tile([128, KCH * TOT_B], DT)
            nc.sync.dma_start(x_sb[:].rearrange("p (c b) -> p c b", c=KCH), xh[:])
            ropec_sb = cpool.tile([128, TOT_B], f32)
            ropes_sb = cpool.tile([128, TOT_B], f32)

            # ---------- phase A: QKV projection ----------
            with tc.tile_pool(name="ps_a", bufs=1, space="PSUM") as ps_a:
                qkv_ps = ps_a.tile([TOT_B, WQKV_W], f32)
                for P in range(4):
                    wq_t = wpool.tile([128, 8 * WQKV_W], DT, tag="wq", bufs=WQ_BUFS)
                    nc.sync.dma_start(
                        wq_t[:].rearrange("p (c j) -> p c j", c=8),
                        wqkv[:, 8 * P:8 * P + 8, :],
                    )
                    if P == 0:
                        nc.scalar.dma_start(ropec_sb[:], ropec[:])
                        nc.scalar.dma_start(ropes_sb[:], ropes[:])
                    for ci in range(8):
                        c = 8 * P + ci
                        lhs = x_sb[:, TOT_B * c:TOT_B * (c + 1)]
                        rhs = wq_t[:, WQKV_W * ci:WQKV_W * (ci + 1)]
                        nc.tensor.matmul(qkv_ps[:, 0:512], lhs, rhs[:, 0:512],
                                         start=(c == 0), stop=(c == KCH - 1))
                        nc.tensor.matmul(qkv_ps[:, 512:768], lhs, rhs[:, 512:768],
                                         start=(c == 0), stop=(c == KCH - 1))

                qkv_sb = cpool.tile([TOT_B, WQKV_W], f32)
                nc.scalar.copy(qkv_sb[:], qkv_ps[:])

            # wo weights prefetch tile. Issued on the SP ring between wqkv and
            # the KV stream: PE's in-order SEQ hits the first group's
            # partial-wo matmuls right after that group's attention, so wo
            # must be resident by ~55us or PE stalls and the second group's
            # whole pipeline backs up. (Not at the very top: a single 4.2MB
            # DMACopy would hold the shared DMA pool ahead of wqkv and delay
            # the QKV projection that gates all attention.)
            wo_all = wpool.tile([128, KCH * QF], DT, tag="wo", bufs=1)
            nc.sync.dma_start(
                wo_all[:].rearrange("p (c j) -> p c j", c=HPC), wo[:])

            # new-position V (plus ones column for the softmax denominator)
            vnew = cpool.tile([TOT_B, 129], DT)
            nc.vector.tensor_copy(vnew[:, 0:HEAD_DIM], qkv_sb[:, 640:768])
            nc.vector.memset(vnew[:, 128:129], 1.0)

            # ---------- transpose q heads + k, apply RoPE ----------
            qT4 = cpool.tile([128, HPC * TOT_B], DT)   # col = b*4 + h
            kTn = cpool.tile([128, TOT_B], DT)         # col = b
            for h in range(HPC + 1):                   # 4 q heads then k
                tp = ps_t.tile([128, TOT_B], f32, tag="tp")
                nc.tensor.transpose(tp[:], qkv_sb[:, 128 * h:128 * (h + 1)],
                                    ident[0:TOT_B, 0:TOT_B])
                t_sb = apool.tile([128, TOT_B], f32, tag="tr")
                nc.vector.tensor_copy(t_sb[:], tp[:])
                sw = apool.tile([128, TOT_B], f32, tag="sw")
                nc.vector.stream_shuffle(sw[:], t_sb[:], SWAP_MASK)
                t1 = apool.tile([128, TOT_B], f32, tag="t1")
                nc.vector.tensor_mul(t1[:], t_sb[:], ropec_sb[:])
                nc.vector.tensor_mul(sw[:], sw[:], ropes_sb[:])
                if h < HPC:
                    dest = qT4[:, h::HPC]
                else:
                    dest = kTn[:]
                nc.vector.tensor_add(dest, t1[:], sw[:])

            # ---------- phase B: attention over the KV cache ----------
            attnT = cpool.tile([128, HPC * TOT_B], DT)  # col = h*32 + b
            kts = (kt0, kt1)
            vps = (vp0, vp1)
            # Output projection is reduction-parallel: each core computes its
            # partial wo product (transposed, [4096, 16] per sample group) and
            # a ReduceScatter(add) sums across cores, leaving each core its
            # own 512 output-feature rows. Group 1 (1024-pos, half the KV
            # bytes) goes FIRST so its collective fires ~54us in, fully
            # overlapped by group 0's KV stream; only group 0's collective
            # sits in the tail. COLLECTIVE_CORES is exclusive, so two
            # back-to-back collectives at the tail would serialize 2x15.8us.
            rs_in = [dpool.tile([DIM, 16], DT, name=f"rs_in{g}") for g in range(2)]
            rs_out = [dpool.tile([QF, 16], DT, name=f"rs_out{g}")
                      for g in range(2)]
            with tc.tile_pool(name="ps_b", bufs=2, space="PSUM") as ps_b:
                for gi, g in enumerate(GROUP_ORDER):
                    npos = SP[g]
                    nf = NFULL[g]
                    ncol = 4 * nf
                    vw = 129 * nf
                    # Small KV slices keep the shared DMA resource's FIFO
                    # shallow: rs_in/y writes and the collectives' inputs
                    # would otherwise queue behind multiple megabyte-sized KV
                    # reads (observed +16us on the first collective). Taper
                    # the tail of the LAST group so the serial per-sample
                    # attention chain after the final DMA byte is short.
                    if SPT == 1:
                        blocks = [1] * BSZ[g]
                    elif gi == 1:
                        blocks = [2] * 7 + [1, 1]
                    else:
                        blocks = [2] * (BSZ[g] // 2)
                    s_off = 0
                    for blk in blocks:
                        ktile = kvpool.tile([128, SPT * SP[0]], DT, tag="kt")
                        nc.sync.dma_start(
                            ktile[:, 0:blk * npos].rearrange("p (s n) -> p s n", s=blk),
                            kts[g][s_off:s_off + blk].rearrange("s p n -> p s n"),
                        )
                        vtile = kvpool.tile([128, SPT * 129 * NFULL[0]], DT, tag="vt")
                        nc.sync.dma_start(
                            vtile[:, 0:blk * vw].rearrange("p (s c d) -> p s c d", s=blk, c=nf),
                            vps[g][s_off:s_off + blk].rearrange("s p c d -> p s c d"),
                        )
                        for j in range(blk):
                            b = 16 * g + s_off + j
                            ks = ktile[:, j * npos:(j + 1) * npos]
                            vs = vtile[:, j * vw:(j + 1) * vw]
                            q_b = qT4[:, HPC * b:HPC * (b + 1)]

                            sc_ps = ps_b.tile([128, 68], f32, tag="sc")
                            for c in range(nf):
                                nc.tensor.matmul(sc_ps[:, 4 * c:4 * c + 4],
                                                 ks[:, 128 * c:128 * (c + 1)], q_b,
                                                 start=True, stop=True)
                            nc.tensor.matmul(sc_ps[0:1, ncol:ncol + 4],
                                             kTn[:, b:b + 1], q_b,
                                             start=True, stop=True)

                            # one exp over the scores plus the new-position
                            # row (cols ncol:ncol+4 rows 1.. hold stale psum
                            # floats that are exp'd but never read)
                            pr = apool.tile([128, 68], DT, tag="pr")
                            nc.scalar.activation(pr[:, 0:ncol + 4],
                                                 sc_ps[:, 0:ncol + 4],
                                                 mybir.ActivationFunctionType.Exp)

                            # select row b of vnew into partition 0 (psum), for the
                            # tail matmul rhs (moving operand must be partition-0 based)
                            vrow_ps = ps_b.tile([1, 129], f32, tag="vr", bufs=1)
                            nc.tensor.matmul(vrow_ps[:], identdt[:, b:b + 1], vnew[:],
                                             start=True, stop=True)
                            vrow = apool.tile([1, 129], DT, tag="vrow")
                            nc.vector.tensor_copy(vrow[:], vrow_ps[:])

                            o_ps = ps_b.tile([HPC, 129], f32, tag="o")
                            for c in range(nf):
                                nc.tensor.matmul(o_ps[:], pr[:, 4 * c:4 * c + 4],
                                                 vs[:, 129 * c:129 * (c + 1)],
                                                 start=(c == 0), stop=False)
                            nc.tensor.matmul(o_ps[:], pr[0:1, ncol:ncol + 4],
                                             vrow[:], start=False, stop=True)

                            rec = apool.tile([HPC, 1], f32, tag="rec")
                            nc.vector.reciprocal(rec[:], o_ps[:, 128:129])
                            at = apool.tile([HPC, HEAD_DIM], f32, tag="at")
                            nc.vector.tensor_scalar_mul(at[:], o_ps[:, 0:HEAD_DIM], rec[:])

                            tp2 = ps_t.tile([128, TOT_B], f32, tag="tp")
                            nc.tensor.transpose(tp2[:, 0:HPC], at[:], ident[0:HPC, 0:HPC])
                            nc.vector.tensor_copy(attnT[:, b::TOT_B], tp2[:, 0:HPC])
                        s_off += blk

                    # this group's samples are done: partial wo product
                    # partialT[f, b] = sum_c wo[f, c] * attn[b, c]  (c = own features)
                    pT_sb = apool.tile([128, 32 * 16], DT, tag="pt", bufs=2)
                    for fq in range(4):          # 8 fb blocks per PSUM bank
                        pt_ps = ps_t.tile([128, 128], f32, tag="tp")
                        for fi in range(8):
                            fb = 8 * fq + fi
                            for h in range(HPC):
                                nc.tensor.matmul(
                                    pt_ps[:, 16 * fi:16 * (fi + 1)],
                                    wo_all[:, h * DIM + 128 * fb:h * DIM + 128 * (fb + 1)],
                                    attnT[:, TOT_B * h + 16 * g:TOT_B * h + 16 * (g + 1)],
                                    start=(h == 0), stop=(h == HPC - 1))
                        nc.vector.tensor_copy(pT_sb[:, 128 * fq:128 * (fq + 1)], pt_ps[:])
                        # rs_in row order is permuted within each rank's
                        # 512-row block (row = 512r + 4p + fb%4) so DRAM
                        # writes are 256B contiguous runs instead of 64B;
                        # host un-permutes (the global mapping rank=fb//4,
                        # row-in-rank=4p+fb%4 is chunk-size invariant).
                        # Shipped in two rank-halves; the tail group's go on
                        # the idle ACT HWDGE ring (faster first-byte than
                        # SWDGE), the first group's on gpsimd mid-stream.
                        if fq == 1 or fq == 3:
                            half = fq // 2
                            dma_eng = nc.scalar if gi == 1 else nc.gpsimd
                            dma_eng.dma_start(
                                rs_in[g][2048 * half:2048 * (half + 1)].rearrange(
                                    "(r p four) b -> p r (four b)", r=NCORE // 2, four=HPC),
                                pT_sb[:, 256 * half:256 * (half + 1)].rearrange(
                                    "p (r four b) -> p r (four b)", r=NCORE // 2, four=HPC),
                            )
                    nc.gpsimd.collective_compute(
                        "ReduceScatter", mybir.AluOpType.add,
                        replica_groups=[list(range(NCORE))],
                        ins=[rs_in[g].opt()], outs=[rs_out[g].opt()],
                    )

                # y writebacks go on the SP ring AFTER both groups' KV
                # dma_starts are queued: a y write waits on its collective,
                # and anything queued behind it on the same in-order ring
                # would stall until the collective completes.
                for g in GROUP_ORDER:
                    nc.sync.dma_start(y[:, 16 * g:16 * (g + 1)], rs_out[g][:])

    nc.finalize()
    return nc


_NC_CACHE = None


def _get_nc():
    global _NC_CACHE
    if _NC_CACHE is None:
        _NC_CACHE = _build_nc()
    return _NC_CACHE


def _prep_inputs(inputs):
    """Shard + lay out the full inputs for the 8 cores."""
    x = np.asarray(inputs["x"], np.float32)
    wq = np.asarray(inputs["wq"], np.float32)
    wk = np.asarray(inputs["wk"], np.float32)
    wv = np.asarray(inputs["wv"], np.float32)
    wo = np.asarray(inputs["wo"], np.float32)
    fc = np.asarray(inputs["freqs_cos"], np.float32)
    fs = np.asarray(inputs["freqs_sin"], np.float32)
    caches = (
        (np.asarray(inputs["cache_k0"], np.float32), np.asarray(inputs["cache_v0"], np.float32)),
        (np.asarray(inputs["cache_k1"], np.float32), np.asarray(inputs["cache_v1"], np.float32)),
    )

    x_flat = x.reshape(TOT_B, DIM)
    xh = np.ascontiguousarray(
        x_flat.T.reshape(KCH, 128, TOT_B).transpose(1, 0, 2)
    ).astype(NPDT)

    # RoPE tables: per-column position (2048 for tokens 0-15, 1024 for 16-31)
    C = np.empty((128, TOT_B), np.float32)
    S = np.empty((128, TOT_B), np.float32)
    for g in range(2):
        cos = fc[SP[g]]
        sin = fs[SP[g]]
        cols = slice(16 * g, 16 * (g + 1))
        C[0::2, cols] = cos[:, None]
        C[1::2, cols] = cos[:, None]
        S[0::2, cols] = -sin[:, None]
        S[1::2, cols] = sin[:, None]

    scale = 1.0 / math.sqrt(HEAD_DIM)

    def _prep_core(r):
        w_q = wq[QF * r:QF * (r + 1)] * scale
        w_k = wk[HEAD_DIM * r:HEAD_DIM * (r + 1)]
        w_v = wv[HEAD_DIM * r:HEAD_DIM * (r + 1)]
        wqkvT = np.concatenate([w_q, w_k, w_v], axis=0).T  # [4096, 768]
        wqkv_hp = np.ascontiguousarray(
            wqkvT.reshape(KCH, 128, 768).transpose(1, 0, 2)
        ).astype(NPDT)

        # wo_cf[local_c, f] = wo[f, 512r + local_c]  -> [128, HPC, 4096]
        wo_cf = wo[:, QF * r:QF * (r + 1)].T  # [512, 4096]
        wo_hp = np.ascontiguousarray(
            wo_cf.reshape(HPC, 128, DIM).transpose(1, 0, 2)
        ).astype(NPDT)

        m = {"xh": xh, "wqkv": wqkv_hp, "wo": wo_hp,
             "ropec": C, "ropes": S}
        for g in range(2):
            ck, cv = caches[g]
            npos = SP[g]
            nf = NFULL[g]
            # cast to the wire dtype first, then do the layout copy at half width
            kslab = ck[:, :npos, r, :].astype(NPDT)       # [16, npos, 128]
            kt = np.ascontiguousarray(kslab.transpose(0, 2, 1))  # [16, 128, npos]
            vslab = cv[:, :npos, r, :].astype(NPDT).reshape(BSZ[g], nf, 128, HEAD_DIM)
            vp = np.empty((BSZ[g], 128, nf, 129), NPDT)
            vp[:, :, :, HEAD_DIM] = NPDT(1.0)
            vp[:, :, :, :HEAD_DIM] = vslab.transpose(0, 2, 1, 3)
            m[f"kt{g}"] = kt
            m[f"vp{g}"] = vp
        return m

    from concurrent.futures import ThreadPoolExecutor
    with ThreadPoolExecutor(max_workers=NCORE) as ex:
        in_maps = list(ex.map(_prep_core, range(NCORE)))
    return in_maps


def _run(inputs, trace=False):
    nc = _get_nc()
    in_maps = _prep_inputs(inputs)
    res = run_bass_kernel_spmd(nc, in_maps, core_ids=list(range(NCORE)), trace=trace)
    # each core returns yT rows [512r : 512r+512] of the [4096, 32] output,
    # row-permuted within the block (row = 4p + fb%4 -> f_local = 128*(fb%4) + p)
    parts = []
    for r in range(NCORE):
        yr = np.asarray(res.results[r]["y"], np.float32)  # [512, 32]
        parts.append(yr.reshape(128, HPC, TOT_B).transpose(1, 0, 2).reshape(QF, TOT_B))
    y_t = np.concatenate(parts, axis=0)
    out = np.ascontiguousarray(y_t.T).reshape(TOT_B, 1, DIM).astype(np.float32)
    return out, res


def kernel(**inputs):
    try:
        out, _ = _run(inputs, trace=False)
    except Exception:
        # transient NRT/axon hiccups have been observed to recover on retry
        out, _ = _run(inputs, trace=False)
    return out



# revision 63
# speedup vs baseline: 1.0468x; 1.0012x over previous
"""Trainium2 Bass kernel for GQA decode attention (nn_Attention_45844480917562).

Tensor-parallel over 8 NeuronCores: each core owns 4 query heads + 1 KV head
(wq/wk/wv column-sharded). The output projection is reduction-parallel: each
core computes its partial wo product transposed and a per-sample-group
ReduceScatter(add) leaves each core its own 512 output-feature rows; the host
only concatenates/transposes.

Compute dtype is bf16 (fp32 PSUM accumulation, fp32 softmax denominator /
division); BASS_ATTN_F32=1 switches to full fp32 at ~2x the HBM traffic.

Self-contained: hardcodes all shapes; host-side prep reshapes/transposes the
full inputs into per-core DMA-friendly layouts (K cache transposed to
[head_dim, pos], V cache chunk-major with a fused ones-column that yields the
softmax denominator for free in the P@V matmul).
"""

import os
import sys
import math

sys.path.insert(0, "/opt/trn_rl_repo")

import numpy as np
import ml_dtypes

import concourse.bass as bass
import concourse.mybir as mybir
from concourse import tile, bacc, masks
from concourse.bass_utils import run_bass_kernel_spmd

# ---------------- problem constants ----------------
DIM = 4096
N_HEADS = 32
N_KV_HEADS = 8
HEAD_DIM = 128
NCORE = 8
HPC = N_HEADS // NCORE            # 4 query heads per core
QF = HPC * HEAD_DIM               # 512 features per core
BSZ = (16, 16)
SP = (2048, 1024)                 # start_pos per group
TOT_B = 32
NFULL = (SP[0] // 128, SP[1] // 128)   # full 128-pos chunks per group: 16, 8
KCH = DIM // 128                  # 32 contraction chunks

USE_F32 = bool(int(os.environ.get("BASS_ATTN_F32", "0")))
# g0 (2048-pos) first: the tail group is then the 1024-pos one, whose
# last-sample attention chain after the final DMA byte is shorter.
GROUP_ORDER = (1, 0) if os.environ.get("BASS_G1_FIRST") else (0, 1)
DT = mybir.dt.float32 if USE_F32 else mybir.dt.bfloat16
NPDT = np.float32 if USE_F32 else ml_dtypes.bfloat16
SPT = 1 if USE_F32 else 2          # samples per KV tile (f32 tiles are 2x bytes)
WQ_BUFS = int(os.environ.get("BASS_WQ_BUFS", "4"))

f32 = mybir.dt.float32


def _build_nc():
    nc = bacc.Bacc(trn_type="TRN2", num_devices=NCORE, enable_asserts=True)

    # ---- I/O ----
    xh = nc.dram_tensor("xh", [128, KCH, TOT_B], DT, kind="ExternalInput")
    wqkv = nc.dram_tensor("wqkv", [128, KCH, QF + 2 * HEAD_DIM], DT, kind="ExternalInput")
    # wo in [local_c, f] layout: wo_cf[p, h, f] = wo[f, 512*r + h*128 + p]
    wo = nc.dram_tensor("wo", [128, HPC, DIM], DT, kind="ExternalInput")
    kt0 = nc.dram_tensor("kt0", [BSZ[0], 128, SP[0]], DT, kind="ExternalInput")
    kt1 = nc.dram_tensor("kt1", [BSZ[1], 128, SP[1]], DT, kind="ExternalInput")
    vp0 = nc.dram_tensor("vp0", [BSZ[0], 128, NFULL[0], 129], DT, kind="ExternalInput")
    vp1 = nc.dram_tensor("vp1", [BSZ[1], 128, NFULL[1], 129], DT, kind="ExternalInput")
    ropec = nc.dram_tensor("ropec", [128, TOT_B], f32, kind="ExternalInput")
    ropes = nc.dram_tensor("ropes", [128, TOT_B], f32, kind="ExternalInput")
    # yT: rows = this core's 512 output features (f = 512*r + row), cols = samples
    # (collectives may not write IO tensors - the BIR verifier rejects it -
    # so ReduceScatter lands in rs_out and a small DMA ships it to y).
    # The whole reduce path runs in the compute dtype (bf16): halves the
    # collective's charged bytes and the rs_in/y DMA traffic; host casts back.
    y = nc.dram_tensor("y", [QF, TOT_B], DT, kind="ExternalOutput")
    DBG = os.environ.get("BASS_DBG_ATTN")
    dbg = (nc.dram_tensor("dbg", [128, HPC * TOT_B], DT, kind="ExternalOutput")
           if DBG else None)
    dbg2 = (nc.dram_tensor("dbg2", [1, HPC * TOT_B], f32, kind="ExternalOutput")
            if DBG == "2" else None)

    WQKV_W = QF + 2 * HEAD_DIM  # 768
    SWAP_MASK = [i ^ 1 for i in range(32)]

    with tile.TileContext(nc) as tc:
        with tc.tile_pool(name="cpool", bufs=1) as cpool, \
             tc.tile_pool(name="wpool", bufs=2) as wpool, \
             tc.tile_pool(name="kvpool", bufs=int(os.environ.get("BASS_KV_BUFS", "3"))) as kvpool, \
             tc.tile_pool(name="apool", bufs=3) as apool, \
             tc.tile_pool(name="ps_t", bufs=2, space="PSUM") as ps_t, \
             tc.tile_pool(name="dpool", bufs=1, space="DRAM") as dpool:

            # ---------- constants ----------
            ident = cpool.tile([128, 128], f32)
            masks.make_identity(nc, ident[:])
            identdt = cpool.tile([TOT_B, TOT_B], DT)
            masks.make_identity(nc, identdt[:])
            ones_row = cpool.tile([1, 128], f32)
            nc.vector.memset(ones_row[:], 1.0)

            # x + wqkv go at the head of the stream so the QKV critical chain
            # gets full DMA bandwidth before the bulk KV traffic. Alternating
            # wqkv chunks between the SP and ACT HWDGE rings pipelines their
            # dispatch/desc-gen latency; rope tables (needed only at ~19us)
            # are issued after chunk 0 to keep them off the critical ramp.
            x_sb = cpool.tile([128, KCH * TOT_B], DT)
            nc.sync.dma_start(x_sb[:].rearrange("p (c b) -> p c b", c=KCH), xh[:])
            ropec_sb = cpool.tile([128, TOT_B], f32)
            ropes_sb = cpool.tile([128, TOT_B], f32)

            # ---------- phase A: QKV projection ----------
            with tc.tile_pool(name="ps_a", bufs=1, space="PSUM") as ps_a:
                qkv_ps = ps_a.tile([TOT_B, WQKV_W], f32)
                for P in range(4):
                    wq_t = wpool.tile([128, 8 * WQKV_W], DT, tag="wq", bufs=WQ_BUFS)
                    nc.sync.dma_start(
                        wq_t[:].rearrange("p (c j) -> p c j", c=8),
                        wqkv[:, 8 * P:8 * P + 8, :],
                    )
                    if P == 0:
                        nc.scalar.dma_start(ropec_sb[:], ropec[:])
                        nc.scalar.dma_start(ropes_sb[:], ropes[:])
                    for ci in range(8):
                        c = 8 * P + ci
                        lhs = x_sb[:, TOT_B * c:TOT_B * (c + 1)]
                        rhs = wq_t[:, WQKV_W * ci:WQKV_W * (ci + 1)]
                        nc.tensor.matmul(qkv_ps[:, 0:512], lhs, rhs[:, 0:512],
                                         start=(c == 0), stop=(c == KCH - 1))
                        nc.tensor.matmul(qkv_ps[:, 512:768], lhs, rhs[:, 512:768],
                                         start=(c == 0), stop=(c == KCH - 1))

                qkv_sb = cpool.tile([TOT_B, WQKV_W], f32)
                nc.scalar.copy(qkv_sb[:], qkv_ps[:])

            # wo weights prefetch tile. Issued on the SP ring between wqkv and
            # the KV stream: PE's in-order SEQ hits the first group's
            # partial-wo matmuls right after that group's attention, so wo
            # must be resident by ~55us or PE stalls and the second group's
            # whole pipeline backs up. (Not at the very top: a single 4.2MB
            # DMACopy would hold the shared DMA pool ahead of wqkv and delay
            # the QKV projection that gates all attention.)
            wo_all = wpool.tile([128, KCH * QF], DT, tag="wo", bufs=1)
            nc.sync.dma_start(
                wo_all[:].rearrange("p (c j) -> p c j", c=HPC), wo[:])

            # new-position V (plus ones column for the softmax denominator)
            vnew = cpool.tile([TOT_B, 129], DT)
            nc.vector.tensor_copy(vnew[:, 0:HEAD_DIM], qkv_sb[:, 640:768])
            nc.vector.memset(vnew[:, 128:129], 1.0)

            # ---------- transpose q heads + k, apply RoPE ----------
            qT4 = cpool.tile([128, HPC * TOT_B], DT)   # col = b*4 + h
            kTn = cpool.tile([128, TOT_B], DT)         # col = b
            for h in range(HPC + 1):                   # 4 q heads then k
                tp = ps_t.tile([128, TOT_B], f32, tag="tp")
                nc.tensor.transpose(tp[:], qkv_sb[:, 128 * h:128 * (h + 1)],
                                    ident[0:TOT_B, 0:TOT_B])
                t_sb = apool.tile([128, TOT_B], f32, tag="tr")
                nc.vector.tensor_copy(t_sb[:], tp[:])
                sw = apool.tile([128, TOT_B], f32, tag="sw")
                nc.vector.stream_shuffle(sw[:], t_sb[:], SWAP_MASK)
                t1 = apool.tile([128, TOT_B], f32, tag="t1")
                nc.vector.tensor_mul(t1[:], t_sb[:], ropec_sb[:])
                nc.vector.tensor_mul(sw[:], sw[:], ropes_sb[:])
                if h < HPC:
                    dest = qT4[:, h::HPC]
                else:
                    dest = kTn[:]
                nc.vector.tensor_add(dest, t1[:], sw[:])

            # ---------- phase B: attention over the KV cache ----------
            attnT = cpool.tile([128, HPC * TOT_B], DT)  # col = h*32 + b
            rec_all = (cpool.tile([1, HPC * TOT_B], f32, name="rec_all")
                       if dbg2 is not None else None)
            kts = (kt0, kt1)
            vps = (vp0, vp1)
            # Output projection is reduction-parallel: each core computes its
            # partial wo product (transposed, [4096, 16] per sample group) and
            # a ReduceScatter(add) sums across cores, leaving each core its
            # own 512 output-feature rows. Group 1 (1024-pos, half the KV
            # bytes) goes FIRST so its collective fires ~54us in, fully
            # overlapped by group 0's KV stream; only group 0's collective
            # sits in the tail. COLLECTIVE_CORES is exclusive, so two
            # back-to-back collectives at the tail would serialize 2x15.8us.
            rs_in = [dpool.tile([DIM, 16], DT, name=f"rs_in{g}") for g in range(2)]
            rs_out = [dpool.tile([QF, 16], DT, name=f"rs_out{g}")
                      for g in range(2)]
            with tc.tile_pool(name="ps_b", bufs=2, space="PSUM") as ps_b:
                for gi, g in enumerate(GROUP_ORDER):
                    npos = SP[g]
                    nf = NFULL[g]
                    ncol = 4 * nf
                    vw = 129 * nf
                    # Small KV slices keep the shared DMA resource's FIFO
                    # shallow: rs_in/y writes and the collectives' inputs
                    # would otherwise queue behind multiple megabyte-sized KV
                    # reads (observed +16us on the first collective). Taper
                    # the tail of the LAST group so the serial per-sample
                    # attention chain after the final DMA byte is short.
                    if SPT == 1:
                        blocks = [1] * BSZ[g]
                    elif gi == 1:
                        blocks = [2] * 7 + [1, 1]
                    else:
                        blocks = [2] * (BSZ[g] // 2)
                    s_off = 0
                    for blk in blocks:
                        ktile = kvpool.tile([128, SPT * SP[0]], DT, tag="kt")
                        nc.sync.dma_start(
                            ktile[:, 0:blk * npos].rearrange("p (s n) -> p s n", s=blk),
                            kts[g][s_off:s_off + blk].rearrange("s p n -> p s n"),
                        )
                        vtile = kvpool.tile([128, SPT * 129 * NFULL[0]], DT, tag="vt")
                        nc.sync.dma_start(
                            vtile[:, 0:blk * vw].rearrange("p (s c d) -> p s c d", s=blk, c=nf),
                            vps[g][s_off:s_off + blk].rearrange("s p c d -> p s c d"),
                        )
                        for j in range(blk):
                            b = 16 * g + s_off + j
                            ks = ktile[:, j * npos:(j + 1) * npos]
                            vs = vtile[:, j * vw:(j + 1) * vw]
                            q_b = qT4[:, HPC * b:HPC * (b + 1)]

                            sc_ps = ps_b.tile([128, 68], f32, tag="sc")
                            for c in range(nf):
                                nc.tensor.matmul(sc_ps[:, 4 * c:4 * c + 4],
                                                 ks[:, 128 * c:128 * (c + 1)], q_b,
                                                 start=True, stop=True)
                            nc.tensor.matmul(sc_ps[0:1, ncol:ncol + 4],
                                             kTn[:, b:b + 1], q_b,
                                             start=True, stop=True)

                            # one exp over the scores plus the new-position
                            # row (cols ncol:ncol+4 rows 1.. hold stale psum
                            # floats that are exp'd but never read)
                            pr = apool.tile([128, 68], DT, tag="pr")
                            nc.scalar.activation(pr[:, 0:ncol + 4],
                                                 sc_ps[:, 0:ncol + 4],
                                                 mybir.ActivationFunctionType.Exp)

                            # select row b of vnew into partition 0 (psum), for the
                            # tail matmul lhs (must be partition-0 based)
                            vrow_ps = ps_b.tile([1, 129], f32, tag="vr", bufs=1)
                            nc.tensor.matmul(vrow_ps[:], identdt[:, b:b + 1], vnew[:],
                                             start=True, stop=True)
                            vrow = apool.tile([1, 129], DT, tag="vrow")
                            nc.vector.tensor_copy(vrow[:], vrow_ps[:])

                            # P@V with V as the stationary operand: the attn
                            # output lands TRANSPOSED ([d, h], the layout the
                            # partial-wo matmul wants) so the per-sample PE
                            # transpose disappears, and the matmul's moving
                            # dim shrinks 129 -> 4. The softmax denominator
                            # accumulates in a parallel [1,4] psum via the
                            # ones column of V.
                            # NOTE: den must live in its OWN psum bank - two
                            # interleaved accumulation chains in one bank
                            # corrupt each other (bank-level accumulate state)
                            o_ps = ps_b.tile([128, HPC], f32, tag="o")
                            den_ps = ps_b.tile([1, HPC], f32, tag="den", bufs=1)
                            for c in range(nf):
                                nc.tensor.matmul(o_ps[:],
                                                 vs[:, 129 * c:129 * c + 128],
                                                 pr[:, 4 * c:4 * c + 4],
                                                 start=(c == 0), stop=False)
                                nc.tensor.matmul(den_ps[:],
                                                 vs[:, 129 * c + 128:129 * (c + 1)],
                                                 pr[:, 4 * c:4 * c + 4],
                                                 start=(c == 0), stop=False)
                            nc.tensor.matmul(o_ps[:], vrow[0:1, 0:128],
                                             pr[0:1, ncol:ncol + 4],
                                             start=False, stop=True)
                            nc.tensor.matmul(den_ps[:],
                                             vrow[0:1, 128:129],
                                             pr[0:1, ncol:ncol + 4],
                                             start=False, stop=True)

                            rec = apool.tile([1, HPC], f32, tag="rec")
                            nc.vector.reciprocal(rec[:], den_ps[:])
                            # broadcast 1/den across partitions (ones outer
                            # product), then scale + store into attnT columns
                            recb_ps = ps_t.tile([128, TOT_B], f32, tag="tp")
                            nc.tensor.matmul(recb_ps[:, 0:HPC], ones_row[:], rec[:],
                                             start=True, stop=True)
                            # DVE may read only one PSUM operand: stage the
                            # broadcast reciprocal in SBUF first
                            recb = apool.tile([128, HPC], f32, tag="rcb")
                            nc.vector.tensor_copy(recb[:], recb_ps[:, 0:HPC])
                            if rec_all is not None:
                                nc.vector.tensor_copy(rec_all[0:1, b::TOT_B], rec[:])
                                nc.vector.tensor_copy(attnT[:, b::TOT_B], o_ps[:])
                            else:
                                nc.vector.tensor_mul(attnT[:, b::TOT_B],
                                                     o_ps[:], recb[:])
                        s_off += blk

                    # this group's samples are done: partial wo product
                    # partialT[f, b] = sum_c wo[f, c] * attn[b, c]  (c = own features)
                    pT_sb = apool.tile([128, 32 * 16], DT, tag="pt", bufs=2)
                    for fq in range(4):          # 8 fb blocks per PSUM bank
                        pt_ps = ps_t.tile([128, 128], f32, tag="tp")
                        for fi in range(8):
                            fb = 8 * fq + fi
                            for h in range(HPC):
                                nc.tensor.matmul(
                                    pt_ps[:, 16 * fi:16 * (fi + 1)],
                                    wo_all[:, h * DIM + 128 * fb:h * DIM + 128 * (fb + 1)],
                                    attnT[:, TOT_B * h + 16 * g:TOT_B * h + 16 * (g + 1)],
                                    start=(h == 0), stop=(h == HPC - 1))
                        nc.vector.tensor_copy(pT_sb[:, 128 * fq:128 * (fq + 1)], pt_ps[:])
                        # rs_in row order is permuted within each rank's
                        # 512-row block (row = 512r + 4p + fb%4) so DRAM
                        # writes are 256B contiguous runs instead of 64B;
                        # host un-permutes (the global mapping rank=fb//4,
                        # row-in-rank=4p+fb%4 is chunk-size invariant).
                        # Shipped in two rank-halves; the tail group's go on
                        # the idle ACT HWDGE ring (faster first-byte than
                        # SWDGE), the first group's on gpsimd mid-stream.
                        if fq == 1 or fq == 3:
                            half = fq // 2
                            dma_eng = nc.scalar if gi == 1 else nc.gpsimd
                            dma_eng.dma_start(
                                rs_in[g][2048 * half:2048 * (half + 1)].rearrange(
                                    "(r p four) b -> p r (four b)", r=NCORE // 2, four=HPC),
                                pT_sb[:, 256 * half:256 * (half + 1)].rearrange(
                                    "p (r four b) -> p r (four b)", r=NCORE // 2, four=HPC),
                            )
                    nc.gpsimd.collective_compute(
                        "ReduceScatter", mybir.AluOpType.add,
                        replica_groups=[list(range(NCORE))],
                        ins=[rs_in[g].opt()], outs=[rs_out[g].opt()],
                    )

                # y writebacks go on the SP ring AFTER both groups' KV
                # dma_starts are queued: a y write waits on its collective,
                # and anything queued behind it on the same in-order ring
                # would stall until the collective completes.
                for g in GROUP_ORDER:
                    nc.sync.dma_start(y[:, 16 * g:16 * (g + 1)], rs_out[g][:])
                if dbg is not None:
                    nc.sync.dma_start(dbg[:], attnT[:])
                if dbg2 is not None:
                    nc.sync.dma_start(dbg2[:], rec_all[:])

    nc.finalize()
    return nc


_NC_CACHE = None


def _get_nc():
    global _NC_CACHE
    if _NC_CACHE is None:
        _NC_CACHE = _build_nc()
    return _NC_CACHE


def _prep_inputs(inputs):
    """Shard + lay out the full inputs for the 8 cores."""
    x = np.asarray(inputs["x"], np.float32)
    wq = np.asarray(inputs["wq"], np.float32)
    wk = np.asarray(inputs["wk"], np.float32)
    wv = np.asarray(inputs["wv"], np.float32)
    wo = np.asarray(inputs["wo"], np.float32)
    fc = np.asarray(inputs["freqs_cos"], np.float32)
    fs = np.asarray(inputs["freqs_sin"], np.float32)
    caches = (
        (np.asarray(inputs["cache_k0"], np.float32), np.asarray(inputs["cache_v0"], np.float32)),
        (np.asarray(inputs["cache_k1"], np.float32), np.asarray(inputs["cache_v1"], np.float32)),
    )

    x_flat = x.reshape(TOT_B, DIM)
    xh = np.ascontiguousarray(
        x_flat.T.reshape(KCH, 128, TOT_B).transpose(1, 0, 2)
    ).astype(NPDT)

    # RoPE tables: per-column position (2048 for tokens 0-15, 1024 for 16-31)
    C = np.empty((128, TOT_B), np.float32)
    S = np.empty((128, TOT_B), np.float32)
    for g in range(2):
        cos = fc[SP[g]]
        sin = fs[SP[g]]
        cols = slice(16 * g, 16 * (g + 1))
        C[0::2, cols] = cos[:, None]
        C[1::2, cols] = cos[:, None]
        S[0::2, cols] = -sin[:, None]
        S[1::2, cols] = sin[:, None]

    scale = 1.0 / math.sqrt(HEAD_DIM)

    def _prep_core(r):
        w_q = wq[QF * r:QF * (r + 1)] * scale
        w_k = wk[HEAD_DIM * r:HEAD_DIM * (r + 1)]
        w_v = wv[HEAD_DIM * r:HEAD_DIM * (r + 1)]
        wqkvT = np.concatenate([w_q, w_k, w_v], axis=0).T  # [4096, 768]
        wqkv_hp = np.ascontiguousarray(
            wqkvT.reshape(KCH, 128, 768).transpose(1, 0, 2)
        ).astype(NPDT)

        # wo_cf[local_c, f] = wo[f, 512r + local_c]  -> [128, HPC, 4096]
        wo_cf = wo[:, QF * r:QF * (r + 1)].T  # [512, 4096]
        wo_hp = np.ascontiguousarray(
            wo_cf.reshape(HPC, 128, DIM).transpose(1, 0, 2)
        ).astype(NPDT)

        m = {"xh": xh, "wqkv": wqkv_hp, "wo": wo_hp,
             "ropec": C, "ropes": S}
        for g in range(2):
            ck, cv = caches[g]
            npos = SP[g]
            nf = NFULL[g]
            # cast to the wire dtype first, then do the layout copy at half width
            kslab = ck[:, :npos, r, :].astype(NPDT)       # [16, npos, 128]
            kt = np.ascontiguousarray(kslab.transpose(0, 2, 1))  # [16, 128, npos]
            vslab = cv[:, :npos, r, :].astype(NPDT).reshape(BSZ[g], nf, 128, HEAD_DIM)
            vp = np.empty((BSZ[g], 128, nf, 129), NPDT)
            vp[:, :, :, HEAD_DIM] = NPDT(1.0)
            vp[:, :, :, :HEAD_DIM] = vslab.transpose(0, 2, 1, 3)
            m[f"kt{g}"] = kt
            m[f"vp{g}"] = vp
        return m

    from concurrent.futures import ThreadPoolExecutor
    with ThreadPoolExecutor(max_workers=NCORE) as ex:
        in_maps = list(ex.map(_prep_core, range(NCORE)))
    return in_maps


def _run(inputs, trace=False):
    nc = _get_nc()
    in_maps = _prep_inputs(inputs)
    res = run_bass_kernel_spmd(nc, in_maps, core_ids=list(range(NCORE)), trace=trace)
    # each core returns yT rows [512r : 512r+512] of the [4096, 32] output,
    # row-permuted within the block (row = 4p + fb%4 -> f_local = 128*(fb%4) + p)
    parts = []
    for r in range(NCORE):
        yr = np.asarray(res.results[r]["y"], np.float32)  # [512, 32]
        parts.append(yr.reshape(128, HPC, TOT_B).transpose(1, 0, 2).reshape(QF, TOT_B))
    y_t = np.concatenate(parts, axis=0)
    out = np.ascontiguousarray(y_t.T).reshape(TOT_B, 1, DIM).astype(np.float32)
    return out, res


def kernel(**inputs):
    try:
        out, _ = _run(inputs, trace=False)
    except Exception:
        # transient NRT/axon hiccups have been observed to recover on retry
        out, _ = _run(inputs, trace=False)
    return out



# revision 64
# speedup vs baseline: 1.0472x; 1.0004x over previous
"""Trainium2 Bass kernel for GQA decode attention (nn_Attention_45844480917562).

Tensor-parallel over 8 NeuronCores: each core owns 4 query heads + 1 KV head
(wq/wk/wv column-sharded). The output projection is reduction-parallel: each
core computes its partial wo product transposed and a per-sample-group
ReduceScatter(add) leaves each core its own 512 output-feature rows; the host
only concatenates/transposes.

Compute dtype is bf16 (fp32 PSUM accumulation, fp32 softmax denominator /
division); BASS_ATTN_F32=1 switches to full fp32 at ~2x the HBM traffic.

Self-contained: hardcodes all shapes; host-side prep reshapes/transposes the
full inputs into per-core DMA-friendly layouts (K cache transposed to
[head_dim, pos], V cache chunk-major with a fused ones-column that yields the
softmax denominator for free in the P@V matmul).
"""

import os
import sys
import math

sys.path.insert(0, "/opt/trn_rl_repo")

import numpy as np
import ml_dtypes

import concourse.bass as bass
import concourse.mybir as mybir
from concourse import tile, bacc, masks
from concourse.bass_utils import run_bass_kernel_spmd

# ---------------- problem constants ----------------
DIM = 4096
N_HEADS = 32
N_KV_HEADS = 8
HEAD_DIM = 128
NCORE = 8
HPC = N_HEADS // NCORE            # 4 query heads per core
QF = HPC * HEAD_DIM               # 512 features per core
BSZ = (16, 16)
SP = (2048, 1024)                 # start_pos per group
TOT_B = 32
NFULL = (SP[0] // 128, SP[1] // 128)   # full 128-pos chunks per group: 16, 8
KCH = DIM // 128                  # 32 contraction chunks

USE_F32 = bool(int(os.environ.get("BASS_ATTN_F32", "0")))
# g0 (2048-pos) first: the tail group is then the 1024-pos one, whose
# last-sample attention chain after the final DMA byte is shorter.
GROUP_ORDER = (1, 0) if os.environ.get("BASS_G1_FIRST") else (0, 1)
DT = mybir.dt.float32 if USE_F32 else mybir.dt.bfloat16
NPDT = np.float32 if USE_F32 else ml_dtypes.bfloat16
SPT = 1 if USE_F32 else 2          # samples per KV tile (f32 tiles are 2x bytes)
WQ_BUFS = int(os.environ.get("BASS_WQ_BUFS", "4"))

f32 = mybir.dt.float32


def _build_nc():
    nc = bacc.Bacc(trn_type="TRN2", num_devices=NCORE, enable_asserts=True)

    # ---- I/O ----
    xh = nc.dram_tensor("xh", [128, KCH, TOT_B], DT, kind="ExternalInput")
    wqkv = nc.dram_tensor("wqkv", [128, KCH, QF + 2 * HEAD_DIM], DT, kind="ExternalInput")
    # wo in [local_c, f] layout: wo_cf[p, h, f] = wo[f, 512*r + h*128 + p]
    wo = nc.dram_tensor("wo", [128, HPC, DIM], DT, kind="ExternalInput")
    kt0 = nc.dram_tensor("kt0", [BSZ[0], 128, SP[0]], DT, kind="ExternalInput")
    kt1 = nc.dram_tensor("kt1", [BSZ[1], 128, SP[1]], DT, kind="ExternalInput")
    vp0 = nc.dram_tensor("vp0", [BSZ[0], 128, NFULL[0], 129], DT, kind="ExternalInput")
    vp1 = nc.dram_tensor("vp1", [BSZ[1], 128, NFULL[1], 129], DT, kind="ExternalInput")
    ropec = nc.dram_tensor("ropec", [128, TOT_B], f32, kind="ExternalInput")
    ropes = nc.dram_tensor("ropes", [128, TOT_B], f32, kind="ExternalInput")
    # yT: rows = this core's 512 output features (f = 512*r + row), cols = samples
    # (collectives may not write IO tensors - the BIR verifier rejects it -
    # so ReduceScatter lands in rs_out and a small DMA ships it to y).
    # The whole reduce path runs in the compute dtype (bf16): halves the
    # collective's charged bytes and the rs_in/y DMA traffic; host casts back.
    y = nc.dram_tensor("y", [QF, TOT_B], DT, kind="ExternalOutput")
    DBG = os.environ.get("BASS_DBG_ATTN")
    dbg = (nc.dram_tensor("dbg", [128, HPC * TOT_B], DT, kind="ExternalOutput")
           if DBG else None)
    dbg2 = (nc.dram_tensor("dbg2", [1, HPC * TOT_B], f32, kind="ExternalOutput")
            if DBG == "2" else None)

    WQKV_W = QF + 2 * HEAD_DIM  # 768
    SWAP_MASK = [i ^ 1 for i in range(32)]

    with tile.TileContext(nc) as tc:
        with tc.tile_pool(name="cpool", bufs=1) as cpool, \
             tc.tile_pool(name="wpool", bufs=2) as wpool, \
             tc.tile_pool(name="kvpool", bufs=int(os.environ.get("BASS_KV_BUFS", "3"))) as kvpool, \
             tc.tile_pool(name="apool", bufs=3) as apool, \
             tc.tile_pool(name="ps_t", bufs=2, space="PSUM") as ps_t, \
             tc.tile_pool(name="dpool", bufs=1, space="DRAM") as dpool:

            # ---------- constants ----------
            ident = cpool.tile([128, 128], f32)
            masks.make_identity(nc, ident[:])
            identdt = cpool.tile([TOT_B, TOT_B], DT)
            masks.make_identity(nc, identdt[:])
            ones_row = cpool.tile([1, 128], f32)
            nc.vector.memset(ones_row[:], 1.0)

            # x + wqkv go at the head of the stream so the QKV critical chain
            # gets full DMA bandwidth before the bulk KV traffic. Alternating
            # wqkv chunks between the SP and ACT HWDGE rings pipelines their
            # dispatch/desc-gen latency; rope tables (needed only at ~19us)
            # are issued after chunk 0 to keep them off the critical ramp.
            x_sb = cpool.tile([128, KCH * TOT_B], DT)
            nc.sync.dma_start(x_sb[:].rearrange("p (c b) -> p c b", c=KCH), xh[:])
            ropec_sb = cpool.tile([128, TOT_B], f32)
            ropes_sb = cpool.tile([128, TOT_B], f32)

            # ---------- phase A: QKV projection ----------
            with tc.tile_pool(name="ps_a", bufs=1, space="PSUM") as ps_a:
                qkv_ps = ps_a.tile([TOT_B, WQKV_W], f32)
                for P in range(4):
                    wq_t = wpool.tile([128, 8 * WQKV_W], DT, tag="wq", bufs=WQ_BUFS)
                    nc.sync.dma_start(
                        wq_t[:].rearrange("p (c j) -> p c j", c=8),
                        wqkv[:, 8 * P:8 * P + 8, :],
                    )
                    if P == 0:
                        nc.scalar.dma_start(ropec_sb[:], ropec[:])
                        nc.scalar.dma_start(ropes_sb[:], ropes[:])
                    for ci in range(8):
                        c = 8 * P + ci
                        lhs = x_sb[:, TOT_B * c:TOT_B * (c + 1)]
                        rhs = wq_t[:, WQKV_W * ci:WQKV_W * (ci + 1)]
                        nc.tensor.matmul(qkv_ps[:, 0:512], lhs, rhs[:, 0:512],
                                         start=(c == 0), stop=(c == KCH - 1))
                        nc.tensor.matmul(qkv_ps[:, 512:768], lhs, rhs[:, 512:768],
                                         start=(c == 0), stop=(c == KCH - 1))

                qkv_sb = cpool.tile([TOT_B, WQKV_W], f32)
                nc.scalar.copy(qkv_sb[:], qkv_ps[:])

            # wo weights prefetch tile. Issued on the SP ring between wqkv and
            # the KV stream: PE's in-order SEQ hits the first group's
            # partial-wo matmuls right after that group's attention, so wo
            # must be resident by ~55us or PE stalls and the second group's
            # whole pipeline backs up. (Not at the very top: a single 4.2MB
            # DMACopy would hold the shared DMA pool ahead of wqkv and delay
            # the QKV projection that gates all attention.)
            wo_all = wpool.tile([128, KCH * QF], DT, tag="wo", bufs=1)
            nc.sync.dma_start(
                wo_all[:].rearrange("p (c j) -> p c j", c=HPC), wo[:])

            # new-position V (plus ones column for the softmax denominator)
            vnew = cpool.tile([TOT_B, 129], DT)
            nc.vector.tensor_copy(vnew[:, 0:HEAD_DIM], qkv_sb[:, 640:768])
            nc.vector.memset(vnew[:, 128:129], 1.0)

            # ---------- transpose q heads + k, apply RoPE ----------
            qT4 = cpool.tile([128, HPC * TOT_B], DT)   # col = b*4 + h
            kTn = cpool.tile([128, TOT_B], DT)         # col = b
            for h in range(HPC + 1):                   # 4 q heads then k
                tp = ps_t.tile([128, TOT_B], f32, tag="tp")
                nc.tensor.transpose(tp[:], qkv_sb[:, 128 * h:128 * (h + 1)],
                                    ident[0:TOT_B, 0:TOT_B])
                t_sb = apool.tile([128, TOT_B], f32, tag="tr")
                nc.vector.tensor_copy(t_sb[:], tp[:])
                sw = apool.tile([128, TOT_B], f32, tag="sw")
                nc.vector.stream_shuffle(sw[:], t_sb[:], SWAP_MASK)
                t1 = apool.tile([128, TOT_B], f32, tag="t1")
                nc.vector.tensor_mul(t1[:], t_sb[:], ropec_sb[:])
                nc.vector.tensor_mul(sw[:], sw[:], ropes_sb[:])
                if h < HPC:
                    dest = qT4[:, h::HPC]
                else:
                    dest = kTn[:]
                nc.vector.tensor_add(dest, t1[:], sw[:])

            # ---------- phase B: attention over the KV cache ----------
            attnT = cpool.tile([128, HPC * TOT_B], DT)  # col = h*32 + b
            rec_all = (cpool.tile([1, HPC * TOT_B], f32, name="rec_all")
                       if dbg2 is not None else None)
            kts = (kt0, kt1)
            vps = (vp0, vp1)
            # Output projection is reduction-parallel: each core computes its
            # partial wo product (transposed, [4096, 16] per sample group) and
            # a ReduceScatter(add) sums across cores, leaving each core its
            # own 512 output-feature rows. Group 1 (1024-pos, half the KV
            # bytes) goes FIRST so its collective fires ~54us in, fully
            # overlapped by group 0's KV stream; only group 0's collective
            # sits in the tail. COLLECTIVE_CORES is exclusive, so two
            # back-to-back collectives at the tail would serialize 2x15.8us.
            rs_in = [dpool.tile([DIM, 16], DT, name=f"rs_in{g}") for g in range(2)]
            rs_out = [dpool.tile([QF, 16], DT, name=f"rs_out{g}")
                      for g in range(2)]
            with tc.tile_pool(name="ps_b", bufs=2, space="PSUM") as ps_b:
                for gi, g in enumerate(GROUP_ORDER):
                    npos = SP[g]
                    nf = NFULL[g]
                    ncol = 4 * nf
                    vw = 129 * nf
                    # Small KV slices keep the shared DMA resource's FIFO
                    # shallow: rs_in/y writes and the collectives' inputs
                    # would otherwise queue behind multiple megabyte-sized KV
                    # reads (observed +16us on the first collective). Taper
                    # the tail of the LAST group so the serial per-sample
                    # attention chain after the final DMA byte is short.
                    if SPT == 1:
                        blocks = [1] * BSZ[g]
                    elif gi == 1:
                        blocks = [2] * 7 + [1, 1]
                    else:
                        blocks = [2] * (BSZ[g] // 2)
                    s_off = 0
                    for blk in blocks:
                        ktile = kvpool.tile([128, SPT * SP[0]], DT, tag="kt")
                        nc.sync.dma_start(
                            ktile[:, 0:blk * npos].rearrange("p (s n) -> p s n", s=blk),
                            kts[g][s_off:s_off + blk].rearrange("s p n -> p s n"),
                        )
                        vtile = kvpool.tile([128, SPT * 129 * NFULL[0]], DT, tag="vt")
                        if gi == 1 and s_off + blk == BSZ[g]:
                            # final sample: V in chunk-halves so the first
                            # P@V matmuls overlap the second half's transfer
                            # (exp gates on ALL scores, so K stays whole)
                            nh = nf // 2
                            for vh in range(2):
                                nc.sync.dma_start(
                                    vtile[:, vh * nh * 129:(vh + 1) * nh * 129]
                                    .rearrange("p (c d) -> p c d", c=nh),
                                    vps[g][s_off:s_off + 1, :, vh * nh:(vh + 1) * nh]
                                    .rearrange("s p c d -> p (s c) d"),
                                )
                        else:
                            nc.sync.dma_start(
                                vtile[:, 0:blk * vw].rearrange("p (s c d) -> p s c d", s=blk, c=nf),
                                vps[g][s_off:s_off + blk].rearrange("s p c d -> p s c d"),
                            )
                        for j in range(blk):
                            b = 16 * g + s_off + j
                            ks = ktile[:, j * npos:(j + 1) * npos]
                            vs = vtile[:, j * vw:(j + 1) * vw]
                            q_b = qT4[:, HPC * b:HPC * (b + 1)]

                            sc_ps = ps_b.tile([128, 68], f32, tag="sc")
                            for c in range(nf):
                                nc.tensor.matmul(sc_ps[:, 4 * c:4 * c + 4],
                                                 ks[:, 128 * c:128 * (c + 1)], q_b,
                                                 start=True, stop=True)
                            nc.tensor.matmul(sc_ps[0:1, ncol:ncol + 4],
                                             kTn[:, b:b + 1], q_b,
                                             start=True, stop=True)

                            # one exp over the scores plus the new-position
                            # row (cols ncol:ncol+4 rows 1.. hold stale psum
                            # floats that are exp'd but never read)
                            pr = apool.tile([128, 68], DT, tag="pr")
                            nc.scalar.activation(pr[:, 0:ncol + 4],
                                                 sc_ps[:, 0:ncol + 4],
                                                 mybir.ActivationFunctionType.Exp)

                            # select row b of vnew into partition 0 (psum), for the
                            # tail matmul lhs (must be partition-0 based)
                            vrow_ps = ps_b.tile([1, 129], f32, tag="vr", bufs=1)
                            nc.tensor.matmul(vrow_ps[:], identdt[:, b:b + 1], vnew[:],
                                             start=True, stop=True)
                            vrow = apool.tile([1, 129], DT, tag="vrow")
                            nc.vector.tensor_copy(vrow[:], vrow_ps[:])

                            # P@V with V as the stationary operand: the attn
                            # output lands TRANSPOSED ([d, h], the layout the
                            # partial-wo matmul wants) so the per-sample PE
                            # transpose disappears, and the matmul's moving
                            # dim shrinks 129 -> 4. The softmax denominator
                            # accumulates in a parallel [1,4] psum via the
                            # ones column of V.
                            # NOTE: den must live in its OWN psum bank - two
                            # interleaved accumulation chains in one bank
                            # corrupt each other (bank-level accumulate state)
                            o_ps = ps_b.tile([128, HPC], f32, tag="o")
                            den_ps = ps_b.tile([1, HPC], f32, tag="den", bufs=1)
                            for c in range(nf):
                                nc.tensor.matmul(o_ps[:],
                                                 vs[:, 129 * c:129 * c + 128],
                                                 pr[:, 4 * c:4 * c + 4],
                                                 start=(c == 0), stop=False)
                                nc.tensor.matmul(den_ps[:],
                                                 vs[:, 129 * c + 128:129 * (c + 1)],
                                                 pr[:, 4 * c:4 * c + 4],
                                                 start=(c == 0), stop=False)
                            nc.tensor.matmul(o_ps[:], vrow[0:1, 0:128],
                                             pr[0:1, ncol:ncol + 4],
                                             start=False, stop=True)
                            nc.tensor.matmul(den_ps[:],
                                             vrow[0:1, 128:129],
                                             pr[0:1, ncol:ncol + 4],
                                             start=False, stop=True)

                            rec = apool.tile([1, HPC], f32, tag="rec")
                            nc.vector.reciprocal(rec[:], den_ps[:])
                            # broadcast 1/den across partitions (ones outer
                            # product), then scale + store into attnT columns
                            recb_ps = ps_t.tile([128, TOT_B], f32, tag="tp")
                            nc.tensor.matmul(recb_ps[:, 0:HPC], ones_row[:], rec[:],
                                             start=True, stop=True)
                            # DVE may read only one PSUM operand: stage the
                            # broadcast reciprocal in SBUF first
                            recb = apool.tile([128, HPC], f32, tag="rcb")
                            nc.vector.tensor_copy(recb[:], recb_ps[:, 0:HPC])
                            if rec_all is not None:
                                nc.vector.tensor_copy(rec_all[0:1, b::TOT_B], rec[:])
                                nc.vector.tensor_copy(attnT[:, b::TOT_B], o_ps[:])
                            else:
                                nc.vector.tensor_mul(attnT[:, b::TOT_B],
                                                     o_ps[:], recb[:])
                        s_off += blk

                    # this group's samples are done: partial wo product
                    # partialT[f, b] = sum_c wo[f, c] * attn[b, c]  (c = own features)
                    pT_sb = apool.tile([128, 32 * 16], DT, tag="pt", bufs=2)
                    for fq in range(4):          # 8 fb blocks per PSUM bank
                        pt_ps = ps_t.tile([128, 128], f32, tag="tp")
                        for fi in range(8):
                            fb = 8 * fq + fi
                            for h in range(HPC):
                                nc.tensor.matmul(
                                    pt_ps[:, 16 * fi:16 * (fi + 1)],
                                    wo_all[:, h * DIM + 128 * fb:h * DIM + 128 * (fb + 1)],
                                    attnT[:, TOT_B * h + 16 * g:TOT_B * h + 16 * (g + 1)],
                                    start=(h == 0), stop=(h == HPC - 1))
                        nc.vector.tensor_copy(pT_sb[:, 128 * fq:128 * (fq + 1)], pt_ps[:])
                        # rs_in row order is permuted within each rank's
                        # 512-row block (row = 512r + 4p + fb%4) so DRAM
                        # writes are 256B contiguous runs instead of 64B;
                        # host un-permutes (the global mapping rank=fb//4,
                        # row-in-rank=4p+fb%4 is chunk-size invariant).
                        # Shipped in two rank-halves; the tail group's go on
                        # the idle ACT HWDGE ring (faster first-byte than
                        # SWDGE), the first group's on gpsimd mid-stream.
                        if fq == 1 or fq == 3:
                            half = fq // 2
                            dma_eng = nc.scalar if gi == 1 else nc.gpsimd
                            dma_eng.dma_start(
                                rs_in[g][2048 * half:2048 * (half + 1)].rearrange(
                                    "(r p four) b -> p r (four b)", r=NCORE // 2, four=HPC),
                                pT_sb[:, 256 * half:256 * (half + 1)].rearrange(
                                    "p (r four b) -> p r (four b)", r=NCORE // 2, four=HPC),
                            )
                    nc.gpsimd.collective_compute(
                        "ReduceScatter", mybir.AluOpType.add,
                        replica_groups=[list(range(NCORE))],
                        ins=[rs_in[g].opt()], outs=[rs_out[g].opt()],
                    )

                # y writebacks go on the SP ring AFTER both groups' KV
                # dma_starts are queued: a y write waits on its collective,
                # and anything queued behind it on the same in-order ring
                # would stall until the collective completes.
                for g in GROUP_ORDER:
                    nc.sync.dma_start(y[:, 16 * g:16 * (g + 1)], rs_out[g][:])
                if dbg is not None:
                    nc.sync.dma_start(dbg[:], attnT[:])
                if dbg2 is not None:
                    nc.sync.dma_start(dbg2[:], rec_all[:])

    nc.finalize()
    return nc


_NC_CACHE = None


def _get_nc():
    global _NC_CACHE
    if _NC_CACHE is None:
        _NC_CACHE = _build_nc()
    return _NC_CACHE


def _prep_inputs(inputs):
    """Shard + lay out the full inputs for the 8 cores."""
    x = np.asarray(inputs["x"], np.float32)
    wq = np.asarray(inputs["wq"], np.float32)
    wk = np.asarray(inputs["wk"], np.float32)
    wv = np.asarray(inputs["wv"], np.float32)
    wo = np.asarray(inputs["wo"], np.float32)
    fc = np.asarray(inputs["freqs_cos"], np.float32)
    fs = np.asarray(inputs["freqs_sin"], np.float32)
    caches = (
        (np.asarray(inputs["cache_k0"], np.float32), np.asarray(inputs["cache_v0"], np.float32)),
        (np.asarray(inputs["cache_k1"], np.float32), np.asarray(inputs["cache_v1"], np.float32)),
    )

    x_flat = x.reshape(TOT_B, DIM)
    xh = np.ascontiguousarray(
        x_flat.T.reshape(KCH, 128, TOT_B).transpose(1, 0, 2)
    ).astype(NPDT)

    # RoPE tables: per-column position (2048 for tokens 0-15, 1024 for 16-31)
    C = np.empty((128, TOT_B), np.float32)
    S = np.empty((128, TOT_B), np.float32)
    for g in range(2):
        cos = fc[SP[g]]
        sin = fs[SP[g]]
        cols = slice(16 * g, 16 * (g + 1))
        C[0::2, cols] = cos[:, None]
        C[1::2, cols] = cos[:, None]
        S[0::2, cols] = -sin[:, None]
        S[1::2, cols] = sin[:, None]

    scale = 1.0 / math.sqrt(HEAD_DIM)

    def _prep_core(r):
        w_q = wq[QF * r:QF * (r + 1)] * scale
        w_k = wk[HEAD_DIM * r:HEAD_DIM * (r + 1)]
        w_v = wv[HEAD_DIM * r:HEAD_DIM * (r + 1)]
        wqkvT = np.concatenate([w_q, w_k, w_v], axis=0).T  # [4096, 768]
        wqkv_hp = np.ascontiguousarray(
            wqkvT.reshape(KCH, 128, 768).transpose(1, 0, 2)
        ).astype(NPDT)

        # wo_cf[local_c, f] = wo[f, 512r + local_c]  -> [128, HPC, 4096]
        wo_cf = wo[:, QF * r:QF * (r + 1)].T  # [512, 4096]
        wo_hp = np.ascontiguousarray(
            wo_cf.reshape(HPC, 128, DIM).transpose(1, 0, 2)
        ).astype(NPDT)

        m = {"xh": xh, "wqkv": wqkv_hp, "wo": wo_hp,
             "ropec": C, "ropes": S}
        for g in range(2):
            ck, cv = caches[g]
            npos = SP[g]
            nf = NFULL[g]
            # cast to the wire dtype first, then do the layout copy at half width
            kslab = ck[:, :npos, r, :].astype(NPDT)       # [16, npos, 128]
            kt = np.ascontiguousarray(kslab.transpose(0, 2, 1))  # [16, 128, npos]
            vslab = cv[:, :npos, r, :].astype(NPDT).reshape(BSZ[g], nf, 128, HEAD_DIM)
            vp = np.empty((BSZ[g], 128, nf, 129), NPDT)
            vp[:, :, :, HEAD_DIM] = NPDT(1.0)
            vp[:, :, :, :HEAD_DIM] = vslab.transpose(0, 2, 1, 3)
            m[f"kt{g}"] = kt
            m[f"vp{g}"] = vp
        return m

    from concurrent.futures import ThreadPoolExecutor
    with ThreadPoolExecutor(max_workers=NCORE) as ex:
        in_maps = list(ex.map(_prep_core, range(NCORE)))
    return in_maps


def _run(inputs, trace=False):
    nc = _get_nc()
    in_maps = _prep_inputs(inputs)
    res = run_bass_kernel_spmd(nc, in_maps, core_ids=list(range(NCORE)), trace=trace)
    # each core returns yT rows [512r : 512r+512] of the [4096, 32] output,
    # row-permuted within the block (row = 4p + fb%4 -> f_local = 128*(fb%4) + p)
    parts = []
    for r in range(NCORE):
        yr = np.asarray(res.results[r]["y"], np.float32)  # [512, 32]
        parts.append(yr.reshape(128, HPC, TOT_B).transpose(1, 0, 2).reshape(QF, TOT_B))
    y_t = np.concatenate(parts, axis=0)
    out = np.ascontiguousarray(y_t.T).reshape(TOT_B, 1, DIM).astype(np.float32)
    return out, res


def kernel(**inputs):
    try:
        out, _ = _run(inputs, trace=False)
    except Exception:
        # transient NRT/axon hiccups have been observed to recover on retry
        out, _ = _run(inputs, trace=False)
    return out

